# revision 62
# baseline (speedup 1.0000x reference)
"""Trainium2 Bass kernel for nn_CausalSelfAttention_74938589380902.

Reference computation (B=4, T=1024, D=1024, H=16, hd=64):
    qkv = x @ w_qkv.T ; split heads
    L   = (q k^T)/8 ; L_y = (q k_y^T)/8  (k_y from separate projection)
    agg = sum(exp(clip(L_y)) * tril) + eps              (per query)
    w   = softplus(log(|L|+eps) - log(agg+eps)) * tril  = log1p((|L|+eps)/(agg+eps)) * tril
    A   = w / (sum(w) + eps) ; out = (A v) merged @ w_proj.T

Sharding: 8 cores = 4 batches x 2 head-groups (8 heads each). Each core
computes its batch/head slice end-to-end and a partial (row-parallel)
projection output, transposed; the host sums the pair of partials per batch.

v2 design (vs the f32r v1 baseline, measured 462us on HW):
  - all matmul operands bf16 (halves DMA, 1 cyc/row even for small matmuls)
  - inputs stream chunk-wise over the SP + ACT HWDGE rings so P1 matmuls
    start ~5us in instead of waiting 45us for one monolithic 4MB DMA
  - per-head pipeline is software-pipelined: Ly matmuls of head h+1 are
    emitted before the binv/broadcast/L-phase of head h, so the PE never
    idles while DVE/ACT finish head h's aggregate
  - abs/mask work moved off DVE onto the idle Pool (gpsimd) engine, with
    the diagonal-tile abs+mask fused into one scalar_tensor_tensor op
  - row-sum reciprocals batched per head-pair ([2,1024] DVE op instead of
    16 single-partition [1,512] ops which cost 2.5us each)
  - merged output written unnormalized, then scaled in place with one
    [128,512] DVE mult per (pair, ic) using a pair-broadcast matmul
  - log1p trick retained: w = Ln(|L_raw| * (0.125/(agg+2eps)) + 1)
  - row sums of w come free from the w@v matmul (ones column in v, M=65)
"""

import sys

sys.path.insert(0, "/opt/trn_rl_repo")

import ml_dtypes
import numpy as np

import concourse.bass as bass
import concourse.mybir as mybir
import concourse.tile as tile
from contextlib import ExitStack

P = 128
T = 1024
D = 1024
B = 4
HEADS_PER_CORE = 8
EPS = 1e-6

_f32 = mybir.dt.float32
_bf16 = mybir.dt.bfloat16
_u16 = mybir.dt.uint16
_u32 = mybir.dt.uint32


def _hi16(ap):
    """High u16 lane of an f32 AP — bf16 payload bits of each float."""
    return ap.bitcast(_u16).rearrange("p (c two) -> p c two", two=2)[:, :, 1]
_AF = mybir.ActivationFunctionType
_OP = mybir.AluOpType
_AX = mybir.AxisListType


def _split_waits(nc, max_waits=1, drain_max=1):
    """Walrus' per-instruction codegen rejects >2 sync-wait commands (the
    Drain CTRL struct rejects >=3; a Matmult S3_LW struct rejected 4). Hoist
    excess waits onto NOPs inserted right before the instruction — the NOP
    blocks the same engine queue, so semantics are preserved."""
    for bb in nc.main_func.blocks:
        idx = 0
        while idx < len(bb.instructions):
            ins = bb.instructions[idx]
            si = ins.sync_info
            if si is None:
                idx += 1
                continue
            limit = drain_max if type(ins).__name__ == "InstDrain" else max_waits
            waits = list(si.on_wait)
            if len(waits) <= limit:
                idx += 1
                continue
            keep, excess = waits[:limit], waits[limit:]
            nops = []
            for i in range(0, len(excess), max_waits):
                nop = mybir.InstNoOp(name=nc.get_next_instruction_name(), ins=[], outs=[])
                nop.engine = ins.engine
                nop.sync_info = mybir.SyncInfo(
                    on_wait=excess[i : i + max_waits], on_update=[]
                )
                nops.append(nop)
            ins.sync_info = mybir.SyncInfo(on_wait=keep, on_update=list(si.on_update))
            for j, nop in enumerate(nops):
                bb.instructions.insert(idx + j, nop)
                nc.register_instruction(nop)
            idx += len(nops) + 1


def build_nc():
    """Build the single-core SPMD program (per-core data arrives as inputs)."""
    nc = bass.Bass()

    xT_d = nc.dram_tensor("xT", [D, T], _bf16, kind="ExternalInput").ap()
    wqk_d = nc.dram_tensor("wqkkT", [D, 1536], _bf16, kind="ExternalInput").ap()
    wvT_d = nc.dram_tensor("wvT", [D, 512], _bf16, kind="ExternalInput").ap()
    wpT_d = nc.dram_tensor("wpT", [512, D], _bf16, kind="ExternalInput").ap()
    identb_d = nc.dram_tensor("identb", [P, P], _bf16, kind="ExternalInput").ap()
    trilm1_d = nc.dram_tensor("trilm1", [P, P], _f32, kind="ExternalInput").ap()
    mtris_d = nc.dram_tensor("mtris", [P, 4, 512], _bf16, kind="ExternalInput").ap()
    e8_d = nc.dram_tensor("e8", [8, 8 * P], _bf16, kind="ExternalInput").ap()
    sel33_d = nc.dram_tensor("sel33", [33, P], _bf16, kind="ExternalInput").ap()
    oT_d = nc.dram_tensor("oT", [D, T], _bf16, kind="ExternalOutput").ap()

    with tile.TileContext(nc) as tc, ExitStack() as ctx:
        # ---- persistent SBUF pools ----
        const_p = ctx.enter_context(tc.tile_pool(name="const", bufs=1))
        qk_p = ctx.enter_context(tc.tile_pool(name="qkky", bufs=1))
        v_p = ctx.enter_context(tc.tile_pool(name="vbuf", bufs=1))
        w_p = ctx.enter_context(tc.tile_pool(name="wbuf", bufs=1))
        mg_p = ctx.enter_context(tc.tile_pool(name="merged", bufs=1))
        x_p = ctx.enter_context(tc.tile_pool(name="xT", bufs=1))
        wv_p = ctx.enter_context(tc.tile_pool(name="wvT", bufs=1))
        wp_p = ctx.enter_context(tc.tile_pool(name="wproj", bufs=1))

        identb = const_p.tile([P, P], _bf16)
        nc.gpsimd.dma_start(identb[:], identb_d[:])
        trilm1 = const_p.tile([P, P], _f32)
        nc.gpsimd.dma_start(trilm1[:], trilm1_d[:])
        mtris = const_p.tile([P, 4, 512], _bf16)
        nc.gpsimd.dma_start(mtris[:], mtris_d[:])
        e8 = const_p.tile([8, 8 * P], _bf16)
        nc.gpsimd.dma_start(e8[:], e8_d[:])
        sel33 = const_p.tile([33, P], _bf16)
        nc.gpsimd.dma_start(sel33[:], sel33_d[:])
        eps16 = const_p.tile([P, 8], _f32)
        nc.gpsimd.memset(eps16[:], 16.0 * EPS)
        # s rows per (parity -> partition 0/32, pair, i); garbage partitions
        # stay 1.0 so the sel33 broadcast matmul sees finite values.
        sall = const_p.tile([33, 4, T], _f32)
        nc.gpsimd.memset(sall[:], 1.0)
        srinv_all = const_p.tile([33, 4, T], _bf16)

        sb_qk = qk_p.tile([P, 12, T], _bf16)  # qT(0-3) kT(4-7) kyT(8-11)
        sb_v = v_p.tile([P, 8, 8, 65], _bf16)  # [t_in, t_blk, head, hd + ones]
        sb_w0 = w_p.tile([P, 8, T], _bf16)  # per-head w (even heads), [j_in, jb, i]
        sb_w1 = w_p.tile([P, 8, T], _bf16)  # per-head w (odd heads)
        sb_mg = mg_p.tile([P, 4, T], _bf16)  # mergedT (A@v; normalized in place)

        # ones column of v via Pool memset (keeps the HWDGE rings for loads);
        # w tiles need no zero-fill — the full-width masked Ln writes every
        # column that the wv matmuls read.
        nc.gpsimd.memset(sb_v[:, :, :, 64], 1.0)

        # ---- streamed input loads (SP + ACT HWDGE rings in parallel) ----
        # Ring order: first wqk tile + x chunks first; late-phase weights
        # (wv, wp) at the back. Each dma_start costs ~1.1us of its ring's
        # sequencer, so chunks are sized to balance latency vs issue count.
        # 8 separate x tiles so each accumulation step waits only its own
        # chunk's DMA (a single tile would gate P1 on the whole 2MB load).
        sb_xc = []
        for dc in range(8):
            xt = x_p.tile([P, T], _bf16, tag=f"xc{dc}")
            sb_xc.append(xt)
        xs = xT_d.rearrange("(dc p) t -> p dc t", p=P)
        ws_p = ctx.enter_context(tc.tile_pool(name="wstream", bufs=1))
        wt_of = {}
        oc_order = [0, 8, 4, 1, 9, 5, 2, 10, 6, 3, 11, 7]
        for i, oc in enumerate(oc_order):
            wt = ws_p.tile([P, 8, P], _bf16, tag=f"wt{oc}")
            wt_of[oc] = wt
        nc.scalar.dma_start(
            wt_of[0][:],
            wqk_d[:, 0:P].rearrange("(dc p) o -> p dc o", p=P),
        )
        for dc in range(8):
            eng = nc.sync if dc % 2 == 0 else nc.scalar
            eng.dma_start(sb_xc[dc][:], xs[:, dc, :])
        for i, oc in enumerate(oc_order[1:]):
            eng = nc.sync if i % 2 == 0 else nc.scalar
            eng.dma_start(
                wt_of[oc][:],
                wqk_d[:, oc * P : (oc + 1) * P].rearrange("(dc p) o -> p dc o", p=P),
            )
        sb_wv = wv_p.tile([P, 8, 512], _bf16)
        nc.sync.dma_start(sb_wv[:], wvT_d.rearrange("(dc p) o -> p dc o", p=P))
        sb_wp = wp_p.tile([P, 4, T], _bf16)  # wpT [i'_in, i'_chunk, c]
        nc.scalar.dma_start(sb_wp[:], wpT_d.rearrange("(kc p) c -> p kc c", p=P))

        # shared psum pools (8 banks: 4 mm + 1 pbq/pinv + 2 pw + 1 ptr)
        mm_p = ctx.enter_context(tc.tile_pool(name="mm_ps", bufs=4, space="PSUM"))
        bc_p = ctx.enter_context(tc.tile_pool(name="bc_ps", bufs=1, space="PSUM"))
        pw_p = ctx.enter_context(tc.tile_pool(name="pw_ps", bufs=2, space="PSUM"))
        ptr_p = ctx.enter_context(tc.tile_pool(name="ptr_ps", bufs=1, space="PSUM"))
        # sbuf scratch pools
        scr_p = ctx.enter_context(tc.tile_pool(name="scr", bufs=2))
        qts_p = ctx.enter_context(tc.tile_pool(name="qts", bufs=3))
        sm_p = ctx.enter_context(tc.tile_pool(name="small", bufs=2))

        def p1_oc(oc):
            """Project one 128-col chunk of q/k/ky into sb_qk (transposed)."""
            wt = wt_of[oc]
            for tn in range(2):
                pt = mm_p.tile([P, 512], _f32, tag="mm")
                for dc in range(8):
                    nc.tensor.matmul(
                        pt[:],
                        lhsT=wt[:, dc, :],
                        rhs=sb_xc[dc][:, tn * 512 : (tn + 1) * 512],
                        start=(dc == 0),
                        stop=(dc == 7),
                    )
                nc.vector.tensor_scalar(
                    sb_qk[:, oc, tn * 512 : (tn + 1) * 512].bitcast(_u16),
                    _hi16(pt[:]),
                    0xFFFF,
                    None,
                    _OP.bitwise_and,
                )

        def p1_v():
            for tb in range(8):
                pt = mm_p.tile([P, 512], _f32, tag="mm")
                for dc in range(8):
                    nc.tensor.matmul(
                        pt[:],
                        lhsT=sb_xc[dc][:, tb * P : (tb + 1) * P],
                        rhs=sb_wv[:, dc, :],
                        start=(dc == 0),
                        stop=(dc == 7),
                    )
                nc.vector.tensor_scalar(
                    sb_v[:, tb, :, 0:64].bitcast(_u16),
                    _hi16(pt[:]).rearrange("p (h e) -> p h e", h=8),
                    0xFFFF,
                    None,
                    _OP.bitwise_and,
                )

        # ---- P2: attention, software-pipelined across heads ----
        aggs_of = {}
        qts_of = {}

        def emit_A1(h):
            """Ly matmuls + exp row-aggregates for head h (PE + ACT + Pool).

            One full-width exp (incl. the unmasked diagonal strip) per
            (b, c) chunk with free-axis accum; the Pool engine then adds
            sum(exp * (tril-1)) over the diagonal strip, subtracting the
            masked-out upper-triangle contribution."""
            qc, po = h // 2, 64 * (h % 2)
            qT = sb_qk[po : po + 64, qc, :]
            kyT = sb_qk[po : po + 64, 8 + qc, :]
            aggs = sm_p.tile([P, 8, 3], _f32, tag="aggs")
            aggs_of[h] = aggs
            nc.gpsimd.memset(aggs[:], 0.0)
            for b in range(8):
                jext = P * b + P
                for c in range(b // 4 + 1):
                    jw = min(512, jext - 512 * c)
                    kw = min(jw, max(0, P * b - 512 * c))
                    ply = mm_p.tile([P, 512], _f32, tag="mm")
                    nc.tensor.matmul(
                        ply[:, :jw],
                        lhsT=qT[:, P * b : P * b + P],
                        rhs=kyT[:, 512 * c : 512 * c + jw],
                        start=True,
                        stop=True,
                    )
                    esc = scr_p.tile([P, 512], _bf16, tag="escr")
                    nc.scalar.activation(
                        esc[:, :jw],
                        ply[:, :jw],
                        _AF.Exp,
                        scale=0.125,
                        accum_out=aggs[:, b, c : c + 1],
                    )
                    if c == b // 4:  # diagonal strip [kw, kw+128)
                        smsk = scr_p.tile([P, P], _bf16, tag="smsk")
                        nc.vector.scalar_tensor_tensor(
                            smsk[:],
                            esc[:, kw : kw + P],
                            1.0,
                            trilm1[:],
                            _OP.mult,
                            _OP.mult,
                            accum_out=aggs[:, b, 2:3],
                        )

        def emit_A2(h):
            """agg -> binv -> broadcast -> scaled q for head h."""
            qc, po = h // 2, 64 * (h % 2)
            qT = sb_qk[po : po + 64, qc, :]
            aggs = aggs_of.pop(h)
            agg8 = sm_p.tile([P, 8], _f32, tag="agg8")
            nc.vector.reduce_sum(agg8[:], aggs[:], axis=_AX.X)
            # b8b = 8*agg + 16eps ; binv = 1/b8b = 0.125/(agg+2eps)
            b8b = sm_p.tile([P, 8], _f32, tag="b8b")
            nc.vector.scalar_tensor_tensor(
                b8b[:], agg8[:], 8.0, eps16[:], _OP.mult, _OP.add
            )
            binv = sm_p.tile([P, 8], _bf16, tag="binv")
            with nc.allow_low_precision(
                reason="bf16 rounding of 1/(8agg+16eps) is ~4e-3 relative"
            ):
                nc.vector.reciprocal(binv[:], b8b[:])
            ptr = ptr_p.tile([8, P], _bf16, tag="ptr")
            nc.tensor.transpose(ptr[:], binv[:], identb[:])
            btr = sm_p.tile([8, P], _bf16, tag="btr")
            nc.vector.tensor_scalar(btr[:], ptr[:], 0.0, None, _OP.add)
            qTs = qts_p.tile([P, T], _bf16, tag="qts")
            qts_of[h] = qTs
            for half in range(2):
                pbq = bc_p.tile([P, 512], _f32, tag="bc")
                for cc in range(4):
                    nc.tensor.matmul(
                        pbq[:, cc * P : (cc + 1) * P],
                        lhsT=e8[:, (4 * half + cc) * P : (4 * half + cc + 1) * P],
                        rhs=btr[:],
                        start=True,
                        stop=True,
                    )
                nc.vector.tensor_tensor(
                    qTs[po : po + 64, 512 * half : 512 * (half + 1)],
                    qT[:, 512 * half : 512 * (half + 1)],
                    pbq[po : po + 64, :],
                    _OP.mult,
                )

        def emit_B(h):
            """L matmuls -> |L|*mask -> Ln -> w@v -> (pair-end) normalize."""
            qc, po = h // 2, 64 * (h % 2)
            kT = sb_qk[po : po + 64, 4 + qc, :]
            qTs = qts_of.pop(h)
            sbw = sb_w0 if h % 2 == 0 else sb_w1
            # L matmuls full-width (masked-out cols are computed then zeroed
            # by the mask — buys uniform tiles and one merged Ln per column
            # half). Diag tiles: DVE |L| via sign-bit clear into f32 scratch,
            # then Pool applies the causal mask (bf16 out). Off-diag tiles:
            # ACT Abs straight to bf16. Ln merged to 2 ACT ops per head.
            for ic in range(2):
                nj = 4 * (ic + 1)
                tw = scr_p.tile([P, 8, 512], _bf16, tag="tw")
                for jb in range(nj):
                    pl = mm_p.tile([P, 512], _f32, tag="mm")
                    nc.tensor.matmul(
                        pl[:],
                        lhsT=kT[:, P * jb : P * jb + P],
                        rhs=qTs[po : po + 64, 512 * ic : 512 * (ic + 1)],
                        start=True,
                        stop=True,
                    )
                    # |L| + f32->bf16 truncation in one 2x-rate DVE op: clear
                    # the sign bit of the high u16 lane of each psum float.
                    nc.vector.tensor_scalar(
                        tw[:, jb, :].bitcast(_u16),
                        _hi16(pl[:]),
                        0x7FFF,
                        None,
                        _OP.bitwise_and,
                    )
                    db = jb - 4 * ic  # diag-block variant, 0..3 when on-diag
                    if 0 <= db <= 3:  # causal mask, bf16 2x-rate DVE
                        nc.vector.tensor_tensor(
                            tw[:, jb, :], tw[:, jb, :], mtris[:, db, :], _OP.mult
                        )
                nc.scalar.activation(
                    sbw[:, 0:nj, 512 * ic : 512 * (ic + 1)],
                    tw[:, 0:nj, :],
                    _AF.Ln,
                    bias=1.0,
                )

            # --- w @ [v | 1]: rows 0-63 = out'^T (unnormalized), row 64 = s_i
            for ic in range(2):
                pw = pw_p.tile([65, 512], _f32, tag="pw")
                nj = 4 * (ic + 1)
                for jb in range(nj):
                    nc.tensor.matmul(
                        pw[:],
                        lhsT=sb_v[:, jb, h, :],
                        rhs=sbw[:, jb, 512 * ic : 512 * (ic + 1)],
                        start=(jb == 0),
                        stop=(jb == nj - 1),
                    )
                # gather s row (+eps) into the pair tile; copy out unnormalized
                pp0 = 32 * (h % 2)
                nc.scalar.activation(
                    sall[pp0 : pp0 + 1, qc, 512 * ic : 512 * (ic + 1)],
                    pw[64:65, :],
                    _AF.Copy,
                    bias=EPS,
                )
                nc.vector.tensor_scalar(
                    sb_mg[po : po + 64, qc, 512 * ic : 512 * (ic + 1)].bitcast(_u16),
                    _hi16(pw[0:64, :]),
                    0xFFFF,
                    None,
                    _OP.bitwise_and,
                )
        def emit_normalize(qc):
            """Deferred pair-end normalize: emitted at the START of the next
            head's block so the reciprocal sits at the front of the DVE
            queue instead of behind a dozen |L| ops (was a 4-6us PE stall
            per pair)."""
            for ic in range(2):
                with nc.allow_low_precision(
                    reason="bf16 rounding of 1/(s+eps) is ~4e-3 relative"
                ):
                    nc.vector.reciprocal(
                        srinv_all[:, qc, 512 * ic : 512 * (ic + 1)],
                        sall[:, qc, 512 * ic : 512 * (ic + 1)],
                    )
                pinv = bc_p.tile([P, 512], _f32, tag="bc")
                nc.tensor.matmul(
                    pinv[:],
                    lhsT=sel33[:],
                    rhs=srinv_all[:, qc, 512 * ic : 512 * (ic + 1)],
                    start=True,
                    stop=True,
                )
                nc.vector.tensor_tensor(
                    sb_mg[:, qc, 512 * ic : 512 * (ic + 1)],
                    sb_mg[:, qc, 512 * ic : 512 * (ic + 1)],
                    pinv[:],
                    _OP.mult,
                )

        # P1 and the Ly/exp aggregate phase interleave per head-pair: the
        # ACT-heavy exp work of all 8 heads hides under P1's matmul burst.
        for p in range(4):
            p1_oc(p)
            p1_oc(8 + p)
            emit_A1(2 * p)
            emit_A1(2 * p + 1)
            p1_oc(4 + p)
        p1_v()

        emit_A2(0)
        pending = None
        for h in range(HEADS_PER_CORE):
            if h + 1 < HEADS_PER_CORE:
                emit_A2(h + 1)
            if pending is not None:
                emit_normalize(pending)
                pending = None
            emit_B(h)
            if h % 2 == 1:
                pending = h // 2
        if pending is not None:
            emit_normalize(pending)

        # ---- P3: project (row-parallel partial), output transposed ----
        with tc.tile_pool(name="obuf", bufs=2) as ob_p:
            for cc in range(8):
                ob = ob_p.tile([P, T], _bf16, tag="ob")
                for tn in range(2):
                    ppj = mm_p.tile([P, 512], _f32, tag="mm")
                    for kc in range(4):
                        nc.tensor.matmul(
                            ppj[:],
                            lhsT=sb_wp[:, kc, cc * P : (cc + 1) * P],
                            rhs=sb_mg[:, kc, tn * 512 : (tn + 1) * 512],
                            start=(kc == 0),
                            stop=(kc == 3),
                        )
                    if tn == 0:
                        nc.vector.tensor_scalar(
                            ob[:, 0:512].bitcast(_u16),
                            _hi16(ppj[:]),
                            0xFFFF,
                            None,
                            _OP.bitwise_and,
                        )
                    else:
                        nc.scalar.copy(ob[:, 512:1024], ppj[:])
                eng = nc.sync if cc % 2 == 0 else nc.scalar
                eng.dma_start(oT_d[cc * P : (cc + 1) * P, :], ob[:])

    _split_waits(nc)
    return nc


_NC_CACHE = None


def _get_nc():
    global _NC_CACHE
    if _NC_CACHE is None:
        _NC_CACHE = build_nc()
    return _NC_CACHE


def shard_inputs(x, w_qkv, w_ky, w_proj):
    """Host-side shard/layout prep. Core c: batch c//2, heads 8*(c%2)..+8."""
    bf = ml_dtypes.bfloat16
    x = np.asarray(x, np.float32)
    w_qkv = np.asarray(w_qkv, np.float32)
    w_ky = np.asarray(w_ky, np.float32)
    w_proj = np.asarray(w_proj, np.float32)

    trilm1 = np.tril(np.ones((P, P), np.float32)) - 1.0
    # mtris[p, v, c] = 1 where key-partition p is causally visible from query
    # column c for a diag-block at column offset 128v: p <= c - 128v.
    mtris = np.zeros((P, 4, 512), bf)
    triu = np.triu(np.ones((P, P), np.float32))
    for v in range(4):
        mtris[:, v, 128 * v : 128 * v + 128] = triu
        mtris[:, v, 128 * v + 128 :] = 1.0
    identb = np.eye(P, dtype=bf)
    e8 = np.zeros((8, 8 * P), bf)
    for cc in range(8):
        e8[cc, cc * P : (cc + 1) * P] = 1.0
    sel33 = np.zeros((33, P), bf)
    sel33[0, 0:64] = 1.0
    sel33[32, 64:128] = 1.0

    in_maps = []
    for c in range(8):
        b, h0 = c // 2, 8 * (c % 2)
        r0 = h0 * 64
        wq = w_qkv[r0 : r0 + 512]
        wk = w_qkv[D + r0 : D + r0 + 512]
        wky = w_ky[r0 : r0 + 512]
        wv = w_qkv[2 * D + r0 : 2 * D + r0 + 512]
        in_maps.append(
            {
                "xT": np.ascontiguousarray(x[b].T).astype(bf),
                "wqkkT": np.ascontiguousarray(
                    np.concatenate([wq, wk, wky], axis=0).T
                ).astype(bf),
                "wvT": np.ascontiguousarray(wv.T).astype(bf),
                "wpT": np.ascontiguousarray(w_proj[:, r0 : r0 + 512].T).astype(bf),
                "identb": identb,
                "trilm1": trilm1,
                "mtris": mtris,
                "e8": e8,
                "sel33": sel33,
            }
        )
    return in_maps


def unshard_output(results):
    """results: list of 8 dicts with 'oT' [D, T] bf16 partials. Sum pairs, transpose."""
    out = np.empty((B, T, D), np.float32)
    for b in range(B):
        acc = results[2 * b]["oT"].astype(np.float32) + results[2 * b + 1][
            "oT"
        ].astype(np.float32)
        out[b] = acc.T
    return out


def kernel(**inputs):
    from concourse.bass_utils import run_bass_kernel_spmd

    nc = _get_nc()
    in_maps = shard_inputs(
        inputs["x"], inputs["w_qkv"], inputs["w_ky"], inputs["w_proj"]
    )
    res = run_bass_kernel_spmd(nc, in_maps, list(range(8)))
    return unshard_output(res.results)


if __name__ == "__main__":
    rng = np.random.default_rng(0)
    ins = {
        "x": rng.normal(size=(B, T, D)).astype(np.float32),
        "w_qkv": rng.normal(size=(3 * D, D)).astype(np.float32) * 0.003,
        "w_ky": rng.normal(size=(D, D)).astype(np.float32) * 0.003,
        "w_proj": rng.normal(size=(D, D)).astype(np.float32) * 0.003,
    }
    out = kernel(**ins)
    print("kernel output", out.shape, out.dtype)


# revision 65
# speedup vs baseline: 1.0054x; 1.0054x over previous
"""Trainium2 Bass kernel for nn_CausalSelfAttention_74938589380902.

Reference computation (B=4, T=1024, D=1024, H=16, hd=64):
    qkv = x @ w_qkv.T ; split heads
    L   = (q k^T)/8 ; L_y = (q k_y^T)/8  (k_y from separate projection)
    agg = sum(exp(clip(L_y)) * tril) + eps              (per query)
    w   = softplus(log(|L|+eps) - log(agg+eps)) * tril  = log1p((|L|+eps)/(agg+eps)) * tril
    A   = w / (sum(w) + eps) ; out = (A v) merged @ w_proj.T

Sharding: 8 cores = 4 batches x 2 head-groups (8 heads each). Each core
computes its batch/head slice end-to-end and a partial (row-parallel)
projection output, transposed; the host sums the pair of partials per batch.

v2 design (vs the f32r v1 baseline, measured 462us on HW):
  - all matmul operands bf16 (halves DMA, 1 cyc/row even for small matmuls)
  - inputs stream chunk-wise over the SP + ACT HWDGE rings so P1 matmuls
    start ~5us in instead of waiting 45us for one monolithic 4MB DMA
  - per-head pipeline is software-pipelined: Ly matmuls of head h+1 are
    emitted before the binv/broadcast/L-phase of head h, so the PE never
    idles while DVE/ACT finish head h's aggregate
  - abs/mask work moved off DVE onto the idle Pool (gpsimd) engine, with
    the diagonal-tile abs+mask fused into one scalar_tensor_tensor op
  - row-sum reciprocals batched per head-pair ([2,1024] DVE op instead of
    16 single-partition [1,512] ops which cost 2.5us each)
  - merged output written unnormalized, then scaled in place with one
    [128,512] DVE mult per (pair, ic) using a pair-broadcast matmul
  - log1p trick retained: w = Ln(|L_raw| * (0.125/(agg+2eps)) + 1)
  - row sums of w come free from the w@v matmul (ones column in v, M=65)
"""

import sys

sys.path.insert(0, "/opt/trn_rl_repo")

import ml_dtypes
import numpy as np

import concourse.bass as bass
import concourse.mybir as mybir
import concourse.tile as tile
from contextlib import ExitStack

P = 128
T = 1024
D = 1024
B = 4
HEADS_PER_CORE = 8
EPS = 1e-6

_f32 = mybir.dt.float32
_bf16 = mybir.dt.bfloat16
_u16 = mybir.dt.uint16
_u32 = mybir.dt.uint32


def _hi16(ap):
    """High u16 lane of an f32 AP — bf16 payload bits of each float."""
    return ap.bitcast(_u16).rearrange("p (c two) -> p c two", two=2)[:, :, 1]
_AF = mybir.ActivationFunctionType
_OP = mybir.AluOpType
_AX = mybir.AxisListType


def _split_waits(nc, max_waits=1, drain_max=1):
    """Walrus' per-instruction codegen rejects >2 sync-wait commands (the
    Drain CTRL struct rejects >=3; a Matmult S3_LW struct rejected 4). Hoist
    excess waits onto NOPs inserted right before the instruction — the NOP
    blocks the same engine queue, so semantics are preserved."""
    for bb in nc.main_func.blocks:
        idx = 0
        while idx < len(bb.instructions):
            ins = bb.instructions[idx]
            si = ins.sync_info
            if si is None:
                idx += 1
                continue
            limit = drain_max if type(ins).__name__ == "InstDrain" else max_waits
            waits = list(si.on_wait)
            if len(waits) <= limit:
                idx += 1
                continue
            keep, excess = waits[:limit], waits[limit:]
            nops = []
            for i in range(0, len(excess), max_waits):
                nop = mybir.InstNoOp(name=nc.get_next_instruction_name(), ins=[], outs=[])
                nop.engine = ins.engine
                nop.sync_info = mybir.SyncInfo(
                    on_wait=excess[i : i + max_waits], on_update=[]
                )
                nops.append(nop)
            ins.sync_info = mybir.SyncInfo(on_wait=keep, on_update=list(si.on_update))
            for j, nop in enumerate(nops):
                bb.instructions.insert(idx + j, nop)
                nc.register_instruction(nop)
            idx += len(nops) + 1


def build_nc():
    """Build the single-core SPMD program (per-core data arrives as inputs)."""
    nc = bass.Bass()

    xT_d = nc.dram_tensor("xT", [D, T], _bf16, kind="ExternalInput").ap()
    wqk_d = nc.dram_tensor("wqkkT", [D, 1536], _bf16, kind="ExternalInput").ap()
    wvT_d = nc.dram_tensor("wvT", [D, 512], _bf16, kind="ExternalInput").ap()
    wpT_d = nc.dram_tensor("wpT", [512, D], _bf16, kind="ExternalInput").ap()
    identb_d = nc.dram_tensor("identb", [P, P], _bf16, kind="ExternalInput").ap()
    trilm1_d = nc.dram_tensor("trilm1", [P, P], _f32, kind="ExternalInput").ap()
    mtris_d = nc.dram_tensor("mtris", [P, 4, 512], _bf16, kind="ExternalInput").ap()
    e8_d = nc.dram_tensor("e8", [8, 8 * P], _bf16, kind="ExternalInput").ap()
    sel33_d = nc.dram_tensor("sel33", [33, P], _bf16, kind="ExternalInput").ap()
    oT_d = nc.dram_tensor("oT", [D, T], _bf16, kind="ExternalOutput").ap()

    with tile.TileContext(nc) as tc, ExitStack() as ctx:
        # ---- persistent SBUF pools ----
        const_p = ctx.enter_context(tc.tile_pool(name="const", bufs=1))
        qk_p = ctx.enter_context(tc.tile_pool(name="qkky", bufs=1))
        v_p = ctx.enter_context(tc.tile_pool(name="vbuf", bufs=1))
        w_p = ctx.enter_context(tc.tile_pool(name="wbuf", bufs=1))
        mg_p = ctx.enter_context(tc.tile_pool(name="merged", bufs=1))
        x_p = ctx.enter_context(tc.tile_pool(name="xT", bufs=1))
        wv_p = ctx.enter_context(tc.tile_pool(name="wvT", bufs=1))
        wp_p = ctx.enter_context(tc.tile_pool(name="wproj", bufs=1))

        identb = const_p.tile([P, P], _bf16)
        nc.gpsimd.dma_start(identb[:], identb_d[:])
        trilm1 = const_p.tile([P, P], _f32)
        nc.gpsimd.dma_start(trilm1[:], trilm1_d[:])
        mtris = const_p.tile([P, 4, 512], _bf16)
        nc.gpsimd.dma_start(mtris[:], mtris_d[:])
        e8 = const_p.tile([8, 8 * P], _bf16)
        nc.gpsimd.dma_start(e8[:], e8_d[:])
        sel33 = const_p.tile([33, P], _bf16)
        nc.gpsimd.dma_start(sel33[:], sel33_d[:])
        eps16 = const_p.tile([P, 8], _f32)
        nc.gpsimd.memset(eps16[:], 16.0 * EPS)
        # s rows per (parity -> partition 0/32, pair, i); garbage partitions
        # stay 1.0 so the sel33 broadcast matmul sees finite values.
        sall = const_p.tile([33, 4, T], _f32)
        nc.gpsimd.memset(sall[:], 1.0)
        srinv_all = const_p.tile([33, 4, T], _bf16)

        sb_qk = qk_p.tile([P, 12, T], _bf16)  # qT(0-3) kT(4-7) kyT(8-11)
        sb_v = v_p.tile([P, 8, 8, 65], _bf16)  # [t_in, t_blk, head, hd + ones]
        sb_w0 = w_p.tile([P, 8, T], _bf16)  # per-head w (even heads), [j_in, jb, i]
        sb_w1 = w_p.tile([P, 8, T], _bf16)  # per-head w (odd heads)
        sb_mg = mg_p.tile([P, 4, T], _bf16)  # mergedT (A@v; normalized in place)

        # ones column of v via Pool memset (keeps the HWDGE rings for loads);
        # w tiles need no zero-fill — the full-width masked Ln writes every
        # column that the wv matmuls read.
        nc.gpsimd.memset(sb_v[:, :, :, 64], 1.0)

        # ---- streamed input loads (SP + ACT HWDGE rings in parallel) ----
        # Ring order: first wqk tile + x chunks first; late-phase weights
        # (wv, wp) at the back. Each dma_start costs ~1.1us of its ring's
        # sequencer, so chunks are sized to balance latency vs issue count.
        # 8 separate x tiles so each accumulation step waits only its own
        # chunk's DMA (a single tile would gate P1 on the whole 2MB load).
        sb_xc = []
        for dc in range(8):
            xt = x_p.tile([P, T], _bf16, tag=f"xc{dc}")
            sb_xc.append(xt)
        xs = xT_d.rearrange("(dc p) t -> p dc t", p=P)
        ws_p = ctx.enter_context(tc.tile_pool(name="wstream", bufs=1))
        wt_of = {}
        oc_order = [0, 8, 4, 1, 9, 5, 2, 10, 6, 3, 11, 7]
        for i, oc in enumerate(oc_order):
            wt = ws_p.tile([P, 8, P], _bf16, tag=f"wt{oc}")
            wt_of[oc] = wt
        nc.scalar.dma_start(
            wt_of[0][:],
            wqk_d[:, 0:P].rearrange("(dc p) o -> p dc o", p=P),
        )
        for dc in range(8):
            eng = nc.sync if dc % 2 == 0 else nc.scalar
            eng.dma_start(sb_xc[dc][:], xs[:, dc, :])
        for i, oc in enumerate(oc_order[1:]):
            eng = nc.sync if i % 2 == 0 else nc.scalar
            eng.dma_start(
                wt_of[oc][:],
                wqk_d[:, oc * P : (oc + 1) * P].rearrange("(dc p) o -> p dc o", p=P),
            )
        sb_wv = wv_p.tile([P, 8, 512], _bf16)
        nc.sync.dma_start(sb_wv[:], wvT_d.rearrange("(dc p) o -> p dc o", p=P))
        sb_wp = wp_p.tile([P, 4, T], _bf16)  # wpT [i'_in, i'_chunk, c]
        nc.scalar.dma_start(sb_wp[:], wpT_d.rearrange("(kc p) c -> p kc c", p=P))

        # shared psum pools (8 banks: 4 mm + 1 pbq/pinv + 2 pw + 1 ptr)
        mm_p = ctx.enter_context(tc.tile_pool(name="mm_ps", bufs=4, space="PSUM"))
        bc_p = ctx.enter_context(tc.tile_pool(name="bc_ps", bufs=1, space="PSUM"))
        pw_p = ctx.enter_context(tc.tile_pool(name="pw_ps", bufs=2, space="PSUM"))
        ptr_p = ctx.enter_context(tc.tile_pool(name="ptr_ps", bufs=1, space="PSUM"))
        # sbuf scratch pools
        scr_p = ctx.enter_context(tc.tile_pool(name="scr", bufs=2))
        qts_p = ctx.enter_context(tc.tile_pool(name="qts", bufs=3))
        sm_p = ctx.enter_context(tc.tile_pool(name="small", bufs=2))

        def p1_oc(oc):
            """Project one 128-col chunk of q/k/ky into sb_qk (transposed)."""
            wt = wt_of[oc]
            for tn in range(2):
                pt = mm_p.tile([P, 512], _f32, tag="mm")
                for dc in range(8):
                    nc.tensor.matmul(
                        pt[:],
                        lhsT=wt[:, dc, :],
                        rhs=sb_xc[dc][:, tn * 512 : (tn + 1) * 512],
                        start=(dc == 0),
                        stop=(dc == 7),
                    )
                nc.vector.tensor_scalar(
                    sb_qk[:, oc, tn * 512 : (tn + 1) * 512].bitcast(_u16),
                    _hi16(pt[:]),
                    0xFFFF,
                    None,
                    _OP.bitwise_and,
                )

        def p1_v():
            for tb in range(8):
                pt = mm_p.tile([P, 512], _f32, tag="mm")
                for dc in range(8):
                    nc.tensor.matmul(
                        pt[:],
                        lhsT=sb_xc[dc][:, tb * P : (tb + 1) * P],
                        rhs=sb_wv[:, dc, :],
                        start=(dc == 0),
                        stop=(dc == 7),
                    )
                nc.vector.tensor_scalar(
                    sb_v[:, tb, :, 0:64].bitcast(_u16),
                    _hi16(pt[:]).rearrange("p (h e) -> p h e", h=8),
                    0xFFFF,
                    None,
                    _OP.bitwise_and,
                )

        # ---- P2: attention, software-pipelined across heads ----
        aggs_of = {}
        qts_of = {}

        def emit_A1(h):
            """Ly matmuls + exp row-aggregates for head h (PE + ACT + Pool).

            One full-width exp (incl. the unmasked diagonal strip) per
            (b, c) chunk with free-axis accum; the Pool engine then adds
            sum(exp * (tril-1)) over the diagonal strip, subtracting the
            masked-out upper-triangle contribution."""
            qc, po = h // 2, 64 * (h % 2)
            qT = sb_qk[po : po + 64, qc, :]
            kyT = sb_qk[po : po + 64, 8 + qc, :]
            aggs = sm_p.tile([P, 8, 3], _f32, tag="aggs")
            aggs_of[h] = aggs
            nc.gpsimd.memset(aggs[:], 0.0)
            for b in range(8):
                jext = P * b + P
                for c in range(b // 4 + 1):
                    jw = min(512, jext - 512 * c)
                    kw = min(jw, max(0, P * b - 512 * c))
                    ply = mm_p.tile([P, 512], _f32, tag="mm")
                    nc.tensor.matmul(
                        ply[:, :jw],
                        lhsT=qT[:, P * b : P * b + P],
                        rhs=kyT[:, 512 * c : 512 * c + jw],
                        start=True,
                        stop=True,
                    )
                    esc = scr_p.tile([P, 512], _bf16, tag="escr")
                    nc.scalar.activation(
                        esc[:, :jw],
                        ply[:, :jw],
                        _AF.Exp,
                        scale=0.125,
                        accum_out=aggs[:, b, c : c + 1],
                    )
                    if c == b // 4:  # diagonal strip [kw, kw+128)
                        smsk = scr_p.tile([P, P], _bf16, tag="smsk")
                        nc.vector.scalar_tensor_tensor(
                            smsk[:],
                            esc[:, kw : kw + P],
                            1.0,
                            trilm1[:],
                            _OP.mult,
                            _OP.mult,
                            accum_out=aggs[:, b, 2:3],
                        )

        def emit_A2(h):
            """agg -> binv -> broadcast -> scaled q for head h."""
            qc, po = h // 2, 64 * (h % 2)
            qT = sb_qk[po : po + 64, qc, :]
            aggs = aggs_of.pop(h)
            agg8 = sm_p.tile([P, 8], _f32, tag="agg8")
            nc.vector.reduce_sum(agg8[:], aggs[:], axis=_AX.X)
            # b8b = 8*agg + 16eps ; binv = 1/b8b = 0.125/(agg+2eps)
            b8b = sm_p.tile([P, 8], _f32, tag="b8b")
            nc.vector.scalar_tensor_tensor(
                b8b[:], agg8[:], 8.0, eps16[:], _OP.mult, _OP.add
            )
            binv = sm_p.tile([P, 8], _bf16, tag="binv")
            with nc.allow_low_precision(
                reason="bf16 rounding of 1/(8agg+16eps) is ~4e-3 relative"
            ):
                nc.vector.reciprocal(binv[:], b8b[:])
            ptr = ptr_p.tile([8, P], _bf16, tag="ptr")
            nc.tensor.transpose(ptr[:], binv[:], identb[:])
            btr = sm_p.tile([8, P], _bf16, tag="btr")
            nc.vector.tensor_scalar(btr[:], ptr[:], 0.0, None, _OP.add)
            qTs = qts_p.tile([P, T], _bf16, tag="qts")
            qts_of[h] = qTs
            for half in range(2):
                pbq = bc_p.tile([P, 512], _f32, tag="bc")
                for cc in range(4):
                    nc.tensor.matmul(
                        pbq[:, cc * P : (cc + 1) * P],
                        lhsT=e8[:, (4 * half + cc) * P : (4 * half + cc + 1) * P],
                        rhs=btr[:],
                        start=True,
                        stop=True,
                    )
                nc.vector.tensor_tensor(
                    qTs[po : po + 64, 512 * half : 512 * (half + 1)],
                    qT[:, 512 * half : 512 * (half + 1)],
                    pbq[po : po + 64, :],
                    _OP.mult,
                )

        def emit_B(h):
            """L matmuls -> |L|*mask -> Ln -> w@v -> (pair-end) normalize."""
            qc, po = h // 2, 64 * (h % 2)
            kT = sb_qk[po : po + 64, 4 + qc, :]
            qTs = qts_of.pop(h)
            sbw = sb_w0 if h % 2 == 0 else sb_w1
            # L matmuls full-width (masked-out cols are computed then zeroed
            # by the mask — buys uniform tiles and one merged Ln per column
            # half). Diag tiles: DVE |L| via sign-bit clear into f32 scratch,
            # then Pool applies the causal mask (bf16 out). Off-diag tiles:
            # ACT Abs straight to bf16. Ln merged to 2 ACT ops per head.
            for ic in range(2):
                nj = 4 * (ic + 1)
                tw = scr_p.tile([P, 8, 512], _bf16, tag="tw")
                for jb in range(nj):
                    pl = mm_p.tile([P, 512], _f32, tag="mm")
                    nc.tensor.matmul(
                        pl[:],
                        lhsT=kT[:, P * jb : P * jb + P],
                        rhs=qTs[po : po + 64, 512 * ic : 512 * (ic + 1)],
                        start=True,
                        stop=True,
                    )
                    # |L| + f32->bf16 truncation in one 2x-rate DVE op: clear
                    # the sign bit of the high u16 lane of each psum float.
                    nc.vector.tensor_scalar(
                        tw[:, jb, :].bitcast(_u16),
                        _hi16(pl[:]),
                        0x7FFF,
                        None,
                        _OP.bitwise_and,
                    )
                    db = jb - 4 * ic  # diag-block variant, 0..3 when on-diag
                    if 0 <= db <= 3:  # causal mask on the otherwise-idle Pool
                        nc.gpsimd.tensor_tensor(
                            tw[:, jb, :], tw[:, jb, :], mtris[:, db, :], _OP.mult
                        )
                nc.scalar.activation(
                    sbw[:, 0:nj, 512 * ic : 512 * (ic + 1)],
                    tw[:, 0:nj, :],
                    _AF.Ln,
                    bias=1.0,
                )

            # --- w @ [v | 1]: rows 0-63 = out'^T (unnormalized), row 64 = s_i
            for ic in range(2):
                pw = pw_p.tile([65, 512], _f32, tag="pw")
                nj = 4 * (ic + 1)
                for jb in range(nj):
                    nc.tensor.matmul(
                        pw[:],
                        lhsT=sb_v[:, jb, h, :],
                        rhs=sbw[:, jb, 512 * ic : 512 * (ic + 1)],
                        start=(jb == 0),
                        stop=(jb == nj - 1),
                    )
                # gather s row (+eps) into the pair tile; copy out unnormalized
                pp0 = 32 * (h % 2)
                nc.scalar.activation(
                    sall[pp0 : pp0 + 1, qc, 512 * ic : 512 * (ic + 1)],
                    pw[64:65, :],
                    _AF.Copy,
                    bias=EPS,
                )
                nc.vector.tensor_scalar(
                    sb_mg[po : po + 64, qc, 512 * ic : 512 * (ic + 1)].bitcast(_u16),
                    _hi16(pw[0:64, :]),
                    0xFFFF,
                    None,
                    _OP.bitwise_and,
                )
            if h % 2 == 1:  # pair end: normalize both heads' merged in place
                with nc.allow_low_precision(
                    reason="bf16 rounding of 1/(s+eps) is ~4e-3 relative"
                ):
                    nc.vector.reciprocal(srinv_all[:, qc, :], sall[:, qc, :])
                for ic in range(2):
                    pinv = bc_p.tile([P, 512], _f32, tag="bc")
                    nc.tensor.matmul(
                        pinv[:],
                        lhsT=sel33[:],
                        rhs=srinv_all[:, qc, 512 * ic : 512 * (ic + 1)],
                        start=True,
                        stop=True,
                    )
                    nc.vector.tensor_tensor(
                        sb_mg[:, qc, 512 * ic : 512 * (ic + 1)],
                        sb_mg[:, qc, 512 * ic : 512 * (ic + 1)],
                        pinv[:],
                        _OP.mult,
                    )

        # P1 and the Ly/exp aggregate phase interleave per head-pair: the
        # ACT-heavy exp work of all 8 heads hides under P1's matmul burst.
        for p in range(4):
            p1_oc(p)
            p1_oc(8 + p)
            emit_A1(2 * p)
            emit_A1(2 * p + 1)
            p1_oc(4 + p)
        p1_v()

        emit_A2(0)
        for h in range(HEADS_PER_CORE):
            if h + 1 < HEADS_PER_CORE:
                emit_A2(h + 1)
            emit_B(h)

        # ---- P3: project (row-parallel partial), output transposed ----
        with tc.tile_pool(name="obuf", bufs=2) as ob_p:
            for cc in range(8):
                ob = ob_p.tile([P, T], _bf16, tag="ob")
                for tn in range(2):
                    ppj = mm_p.tile([P, 512], _f32, tag="mm")
                    for kc in range(4):
                        nc.tensor.matmul(
                            ppj[:],
                            lhsT=sb_wp[:, kc, cc * P : (cc + 1) * P],
                            rhs=sb_mg[:, kc, tn * 512 : (tn + 1) * 512],
                            start=(kc == 0),
                            stop=(kc == 3),
                        )
                    if tn == 0:
                        nc.vector.tensor_scalar(
                            ob[:, 0:512].bitcast(_u16),
                            _hi16(ppj[:]),
                            0xFFFF,
                            None,
                            _OP.bitwise_and,
                        )
                    else:
                        nc.scalar.copy(ob[:, 512:1024], ppj[:])
                eng = nc.sync if cc % 2 == 0 else nc.scalar
                eng.dma_start(oT_d[cc * P : (cc + 1) * P, :], ob[:])

    _split_waits(nc)
    return nc


_NC_CACHE = None


def _get_nc():
    global _NC_CACHE
    if _NC_CACHE is None:
        _NC_CACHE = build_nc()
    return _NC_CACHE


def shard_inputs(x, w_qkv, w_ky, w_proj):
    """Host-side shard/layout prep. Core c: batch c//2, heads 8*(c%2)..+8."""
    bf = ml_dtypes.bfloat16
    x = np.asarray(x, np.float32)
    w_qkv = np.asarray(w_qkv, np.float32)
    w_ky = np.asarray(w_ky, np.float32)
    w_proj = np.asarray(w_proj, np.float32)

    trilm1 = np.tril(np.ones((P, P), np.float32)) - 1.0
    # mtris[p, v, c] = 1 where key-partition p is causally visible from query
    # column c for a diag-block at column offset 128v: p <= c - 128v.
    mtris = np.zeros((P, 4, 512), bf)
    triu = np.triu(np.ones((P, P), np.float32))
    for v in range(4):
        mtris[:, v, 128 * v : 128 * v + 128] = triu
        mtris[:, v, 128 * v + 128 :] = 1.0
    identb = np.eye(P, dtype=bf)
    e8 = np.zeros((8, 8 * P), bf)
    for cc in range(8):
        e8[cc, cc * P : (cc + 1) * P] = 1.0
    sel33 = np.zeros((33, P), bf)
    sel33[0, 0:64] = 1.0
    sel33[32, 64:128] = 1.0

    in_maps = []
    for c in range(8):
        b, h0 = c // 2, 8 * (c % 2)
        r0 = h0 * 64
        wq = w_qkv[r0 : r0 + 512]
        wk = w_qkv[D + r0 : D + r0 + 512]
        wky = w_ky[r0 : r0 + 512]
        wv = w_qkv[2 * D + r0 : 2 * D + r0 + 512]
        in_maps.append(
            {
                "xT": np.ascontiguousarray(x[b].T).astype(bf),
                "wqkkT": np.ascontiguousarray(
                    np.concatenate([wq, wk, wky], axis=0).T
                ).astype(bf),
                "wvT": np.ascontiguousarray(wv.T).astype(bf),
                "wpT": np.ascontiguousarray(w_proj[:, r0 : r0 + 512].T).astype(bf),
                "identb": identb,
                "trilm1": trilm1,
                "mtris": mtris,
                "e8": e8,
                "sel33": sel33,
            }
        )
    return in_maps


def unshard_output(results):
    """results: list of 8 dicts with 'oT' [D, T] bf16 partials. Sum pairs, transpose."""
    out = np.empty((B, T, D), np.float32)
    for b in range(B):
        acc = results[2 * b]["oT"].astype(np.float32) + results[2 * b + 1][
            "oT"
        ].astype(np.float32)
        out[b] = acc.T
    return out


def kernel(**inputs):
    from concourse.bass_utils import run_bass_kernel_spmd

    nc = _get_nc()
    in_maps = shard_inputs(
        inputs["x"], inputs["w_qkv"], inputs["w_ky"], inputs["w_proj"]
    )
    res = run_bass_kernel_spmd(nc, in_maps, list(range(8)))
    return unshard_output(res.results)


if __name__ == "__main__":
    rng = np.random.default_rng(0)
    ins = {
        "x": rng.normal(size=(B, T, D)).astype(np.float32),
        "w_qkv": rng.normal(size=(3 * D, D)).astype(np.float32) * 0.003,
        "w_ky": rng.normal(size=(D, D)).astype(np.float32) * 0.003,
        "w_proj": rng.normal(size=(D, D)).astype(np.float32) * 0.003,
    }
    out = kernel(**ins)
    print("kernel output", out.shape, out.dtype)


# revision 68
# speedup vs baseline: 1.1684x; 1.1621x over previous
"""Trainium2 Bass kernel for nn_CausalSelfAttention_74938589380902.

Reference computation (B=4, T=1024, D=1024, H=16, hd=64):
    qkv = x @ w_qkv.T ; split heads
    L   = (q k^T)/8 ; L_y = (q k_y^T)/8  (k_y from separate projection)
    agg = sum(exp(clip(L_y)) * tril) + eps              (per query)
    w   = softplus(log(|L|+eps) - log(agg+eps)) * tril  = log1p((|L|+eps)/(agg+eps)) * tril
    A   = w / (sum(w) + eps) ; out = (A v) merged @ w_proj.T

Sharding: 8 cores = 4 batches x 2 head-groups (8 heads each). Each core
computes its batch/head slice end-to-end and a partial (row-parallel)
projection output, transposed; the host sums the pair of partials per batch.

v2 design (vs the f32r v1 baseline, measured 462us on HW):
  - all matmul operands bf16 (halves DMA, 1 cyc/row even for small matmuls)
  - inputs stream chunk-wise over the SP + ACT HWDGE rings so P1 matmuls
    start ~5us in instead of waiting 45us for one monolithic 4MB DMA
  - per-head pipeline is software-pipelined: Ly matmuls of head h+1 are
    emitted before the binv/broadcast/L-phase of head h, so the PE never
    idles while DVE/ACT finish head h's aggregate
  - abs/mask work moved off DVE onto the idle Pool (gpsimd) engine, with
    the diagonal-tile abs+mask fused into one scalar_tensor_tensor op
  - row-sum reciprocals batched per head-pair ([2,1024] DVE op instead of
    16 single-partition [1,512] ops which cost 2.5us each)
  - merged output written unnormalized, then scaled in place with one
    [128,512] DVE mult per (pair, ic) using a pair-broadcast matmul
  - log1p trick retained: w = Ln(|L_raw| * (0.125/(agg+2eps)) + 1)
  - row sums of w come free from the w@v matmul (ones column in v, M=65)
"""

import sys

sys.path.insert(0, "/opt/trn_rl_repo")

import ml_dtypes
import numpy as np

import concourse.bass as bass
import concourse.mybir as mybir
import concourse.tile as tile
from contextlib import ExitStack

P = 128
T = 1024
D = 1024
B = 4
HEADS_PER_CORE = 8
EPS = 1e-6

_f32 = mybir.dt.float32
_bf16 = mybir.dt.bfloat16
_u16 = mybir.dt.uint16
_u32 = mybir.dt.uint32


def _hi16(ap):
    """High u16 lane of an f32 AP — bf16 payload bits of each float."""
    return ap.bitcast(_u16).rearrange("p (c two) -> p c two", two=2)[:, :, 1]
_AF = mybir.ActivationFunctionType
_OP = mybir.AluOpType
_AX = mybir.AxisListType


def _split_waits(nc, max_waits=1, drain_max=1):
    """Walrus' per-instruction codegen rejects >2 sync-wait commands (the
    Drain CTRL struct rejects >=3; a Matmult S3_LW struct rejected 4). Hoist
    excess waits onto NOPs inserted right before the instruction — the NOP
    blocks the same engine queue, so semantics are preserved."""
    for bb in nc.main_func.blocks:
        idx = 0
        while idx < len(bb.instructions):
            ins = bb.instructions[idx]
            si = ins.sync_info
            if si is None:
                idx += 1
                continue
            limit = drain_max if type(ins).__name__ == "InstDrain" else max_waits
            waits = list(si.on_wait)
            if len(waits) <= limit:
                idx += 1
                continue
            keep, excess = waits[:limit], waits[limit:]
            nops = []
            for i in range(0, len(excess), max_waits):
                nop = mybir.InstNoOp(name=nc.get_next_instruction_name(), ins=[], outs=[])
                nop.engine = ins.engine
                nop.sync_info = mybir.SyncInfo(
                    on_wait=excess[i : i + max_waits], on_update=[]
                )
                nops.append(nop)
            ins.sync_info = mybir.SyncInfo(on_wait=keep, on_update=list(si.on_update))
            for j, nop in enumerate(nops):
                bb.instructions.insert(idx + j, nop)
                nc.register_instruction(nop)
            idx += len(nops) + 1


def build_nc():
    """Build the single-core SPMD program (per-core data arrives as inputs)."""
    nc = bass.Bass()

    xT_d = nc.dram_tensor("xT", [D, T], _bf16, kind="ExternalInput").ap()
    wqk_d = nc.dram_tensor("wqkkT", [D, 1536], _bf16, kind="ExternalInput").ap()
    wvT_d = nc.dram_tensor("wvT", [D, 512], _bf16, kind="ExternalInput").ap()
    wpT_d = nc.dram_tensor("wpT", [512, D], _bf16, kind="ExternalInput").ap()
    identb_d = nc.dram_tensor("identb", [P, P], _bf16, kind="ExternalInput").ap()
    trilm1_d = nc.dram_tensor("trilm1", [P, P], _f32, kind="ExternalInput").ap()
    mtris_d = nc.dram_tensor("mtris", [P, 4, 512], _bf16, kind="ExternalInput").ap()
    e8_d = nc.dram_tensor("e8", [8, 8 * P], _bf16, kind="ExternalInput").ap()
    sel33_d = nc.dram_tensor("sel33", [33, P], _bf16, kind="ExternalInput").ap()
    oT_d = nc.dram_tensor("oT", [D, T], _bf16, kind="ExternalOutput").ap()

    with tile.TileContext(nc) as tc, ExitStack() as ctx:
        # ---- persistent SBUF pools ----
        const_p = ctx.enter_context(tc.tile_pool(name="const", bufs=1))
        qk_p = ctx.enter_context(tc.tile_pool(name="qkky", bufs=1))
        v_p = ctx.enter_context(tc.tile_pool(name="vbuf", bufs=1))
        w_p = ctx.enter_context(tc.tile_pool(name="wbuf", bufs=1))
        mg_p = ctx.enter_context(tc.tile_pool(name="merged", bufs=1))
        x_p = ctx.enter_context(tc.tile_pool(name="xT", bufs=1))
        wv_p = ctx.enter_context(tc.tile_pool(name="wvT", bufs=1))
        wp_p = ctx.enter_context(tc.tile_pool(name="wproj", bufs=1))

        identb = const_p.tile([P, P], _bf16)
        nc.gpsimd.dma_start(identb[:], identb_d[:])
        trilm1 = const_p.tile([P, P], _f32)
        nc.gpsimd.dma_start(trilm1[:], trilm1_d[:])
        mtris = const_p.tile([P, 4, 512], _bf16)
        nc.gpsimd.dma_start(mtris[:], mtris_d[:])
        e8 = const_p.tile([8, 8 * P], _bf16)
        nc.gpsimd.dma_start(e8[:], e8_d[:])
        sel33 = const_p.tile([33, P], _bf16)
        nc.gpsimd.dma_start(sel33[:], sel33_d[:])
        eps16 = const_p.tile([P, 8], _f32)
        nc.gpsimd.memset(eps16[:], 16.0 * EPS)
        # s rows per (parity -> partition 0/32, pair, i); garbage partitions
        # stay 1.0 so the sel33 broadcast matmul sees finite values.
        sall = const_p.tile([33, 4, T], _f32)
        nc.gpsimd.memset(sall[:], 1.0)
        srinv_all = const_p.tile([33, 4, T], _bf16)

        sb_qk = qk_p.tile([P, 12, T], _bf16)  # qT(0-3) kT(4-7) kyT(8-11)
        sb_v = v_p.tile([P, 8, 8, 65], _bf16)  # [t_in, t_blk, head, hd + ones]
        sb_w0 = w_p.tile([P, 8, T], _bf16)  # per-head w (even heads), [j_in, jb, i]
        sb_w1 = w_p.tile([P, 8, T], _bf16)  # per-head w (odd heads)
        sb_mg = mg_p.tile([P, 4, T], _bf16)  # mergedT (A@v; normalized in place)

        # ones column of v via Pool memset (keeps the HWDGE rings for loads);
        # w tiles need no zero-fill — the full-width masked Ln writes every
        # column that the wv matmuls read.
        nc.gpsimd.memset(sb_v[:, :, :, 64], 1.0)

        # ---- streamed input loads (SP + ACT HWDGE rings in parallel) ----
        # Ring order: first wqk tile + x chunks first; late-phase weights
        # (wv, wp) at the back. Each dma_start costs ~1.1us of its ring's
        # sequencer, so chunks are sized to balance latency vs issue count.
        # 8 separate x tiles so each accumulation step waits only its own
        # chunk's DMA (a single tile would gate P1 on the whole 2MB load).
        sb_xc = []
        for dc in range(8):
            xt = x_p.tile([P, T], _bf16, tag=f"xc{dc}")
            sb_xc.append(xt)
        xs = xT_d.rearrange("(dc p) t -> p dc t", p=P)
        ws_p = ctx.enter_context(tc.tile_pool(name="wstream", bufs=1))
        wt_of = {}
        oc_order = [0, 8, 4, 1, 9, 5, 2, 10, 6, 3, 11, 7]
        for i, oc in enumerate(oc_order):
            wt = ws_p.tile([P, 8, P], _bf16, tag=f"wt{oc}")
            wt_of[oc] = wt
        nc.scalar.dma_start(
            wt_of[0][:],
            wqk_d[:, 0:P].rearrange("(dc p) o -> p dc o", p=P),
        )
        for dc in range(8):
            eng = nc.sync if dc % 2 == 0 else nc.scalar
            eng.dma_start(sb_xc[dc][:], xs[:, dc, :])
        for i, oc in enumerate(oc_order[1:]):
            eng = nc.sync if i % 2 == 0 else nc.scalar
            eng.dma_start(
                wt_of[oc][:],
                wqk_d[:, oc * P : (oc + 1) * P].rearrange("(dc p) o -> p dc o", p=P),
            )
        sb_wv = wv_p.tile([P, 8, 512], _bf16)
        nc.sync.dma_start(sb_wv[:], wvT_d.rearrange("(dc p) o -> p dc o", p=P))
        sb_wp = wp_p.tile([P, 4, T], _bf16)  # wpT [i'_in, i'_chunk, c]
        nc.scalar.dma_start(sb_wp[:], wpT_d.rearrange("(kc p) c -> p kc c", p=P))

        # shared psum pools (8 banks: 3 mm + 1 pbq + 1 pinv + 2 pw + 1 ptr)
        mm_p = ctx.enter_context(tc.tile_pool(name="mm_ps", bufs=3, space="PSUM"))
        bc_p = ctx.enter_context(tc.tile_pool(name="bc_ps", bufs=1, space="PSUM"))
        pi_p = ctx.enter_context(tc.tile_pool(name="pi_ps", bufs=1, space="PSUM"))
        pw_p = ctx.enter_context(tc.tile_pool(name="pw_ps", bufs=2, space="PSUM"))
        ptr_p = ctx.enter_context(tc.tile_pool(name="ptr_ps", bufs=1, space="PSUM"))
        # sbuf scratch pools
        scr_p = ctx.enter_context(tc.tile_pool(name="scr", bufs=2))
        qts_p = ctx.enter_context(tc.tile_pool(name="qts", bufs=3))
        sm_p = ctx.enter_context(tc.tile_pool(name="small", bufs=2))

        def p1_oc(oc):
            """Project one 128-col chunk of q/k/ky into sb_qk (transposed)."""
            wt = wt_of[oc]
            for tn in range(2):
                pt = mm_p.tile([P, 512], _f32, tag="mm")
                for dc in range(8):
                    nc.tensor.matmul(
                        pt[:],
                        lhsT=wt[:, dc, :],
                        rhs=sb_xc[dc][:, tn * 512 : (tn + 1) * 512],
                        start=(dc == 0),
                        stop=(dc == 7),
                    )
                nc.vector.tensor_scalar(
                    sb_qk[:, oc, tn * 512 : (tn + 1) * 512].bitcast(_u16),
                    _hi16(pt[:]),
                    0xFFFF,
                    None,
                    _OP.bitwise_and,
                )

        def p1_v():
            for tb in range(8):
                pt = mm_p.tile([P, 512], _f32, tag="mm")
                for dc in range(8):
                    nc.tensor.matmul(
                        pt[:],
                        lhsT=sb_xc[dc][:, tb * P : (tb + 1) * P],
                        rhs=sb_wv[:, dc, :],
                        start=(dc == 0),
                        stop=(dc == 7),
                    )
                nc.vector.tensor_scalar(
                    sb_v[:, tb, :, 0:64].bitcast(_u16),
                    _hi16(pt[:]).rearrange("p (h e) -> p h e", h=8),
                    0xFFFF,
                    None,
                    _OP.bitwise_and,
                )

        # ---- P2: attention, software-pipelined across heads ----
        aggs_of = {}
        qts_of = {}

        def emit_A1(h):
            """Ly matmuls + exp row-aggregates for head h (PE + ACT + Pool).

            One full-width exp (incl. the unmasked diagonal strip) per
            (b, c) chunk with free-axis accum; the Pool engine then adds
            sum(exp * (tril-1)) over the diagonal strip, subtracting the
            masked-out upper-triangle contribution."""
            qc, po = h // 2, 64 * (h % 2)
            qT = sb_qk[po : po + 64, qc, :]
            kyT = sb_qk[po : po + 64, 8 + qc, :]
            aggs = sm_p.tile([P, 8, 3], _f32, tag="aggs")
            aggs_of[h] = aggs
            nc.gpsimd.memset(aggs[:], 0.0)
            for b in range(8):
                jext = P * b + P
                for c in range(b // 4 + 1):
                    jw = min(512, jext - 512 * c)
                    kw = min(jw, max(0, P * b - 512 * c))
                    ply = mm_p.tile([P, 512], _f32, tag="mm")
                    nc.tensor.matmul(
                        ply[:, :jw],
                        lhsT=qT[:, P * b : P * b + P],
                        rhs=kyT[:, 512 * c : 512 * c + jw],
                        start=True,
                        stop=True,
                    )
                    esc = scr_p.tile([P, 512], _bf16, tag="escr")
                    nc.scalar.activation(
                        esc[:, :jw],
                        ply[:, :jw],
                        _AF.Exp,
                        scale=0.125,
                        accum_out=aggs[:, b, c : c + 1],
                    )
                    if c == b // 4:  # diagonal strip [kw, kw+128)
                        smsk = scr_p.tile([P, P], _bf16, tag="smsk")
                        nc.vector.scalar_tensor_tensor(
                            smsk[:],
                            esc[:, kw : kw + P],
                            1.0,
                            trilm1[:],
                            _OP.mult,
                            _OP.mult,
                            accum_out=aggs[:, b, 2:3],
                        )

        def emit_A2(h):
            """agg -> binv -> broadcast -> scaled q for head h."""
            qc, po = h // 2, 64 * (h % 2)
            qT = sb_qk[po : po + 64, qc, :]
            aggs = aggs_of.pop(h)
            agg8 = sm_p.tile([P, 8], _f32, tag="agg8")
            nc.vector.reduce_sum(agg8[:], aggs[:], axis=_AX.X)
            # b8b = 8*agg + 16eps ; binv = 1/b8b = 0.125/(agg+2eps)
            b8b = sm_p.tile([P, 8], _f32, tag="b8b")
            nc.vector.scalar_tensor_tensor(
                b8b[:], agg8[:], 8.0, eps16[:], _OP.mult, _OP.add
            )
            binv = sm_p.tile([P, 8], _bf16, tag="binv")
            with nc.allow_low_precision(
                reason="bf16 rounding of 1/(8agg+16eps) is ~4e-3 relative"
            ):
                nc.vector.reciprocal(binv[:], b8b[:])
            ptr = ptr_p.tile([8, P], _bf16, tag="ptr")
            nc.tensor.transpose(ptr[:], binv[:], identb[:])
            btr = sm_p.tile([8, P], _bf16, tag="btr")
            nc.vector.tensor_scalar(btr[:], ptr[:], 0.0, None, _OP.add)
            qTs = qts_p.tile([P, T], _bf16, tag="qts")
            qts_of[h] = qTs
            for half in range(2):
                pbq = bc_p.tile([P, 512], _f32, tag="bc")
                for cc in range(4):
                    nc.tensor.matmul(
                        pbq[:, cc * P : (cc + 1) * P],
                        lhsT=e8[:, (4 * half + cc) * P : (4 * half + cc + 1) * P],
                        rhs=btr[:],
                        start=True,
                        stop=True,
                    )
                nc.vector.tensor_tensor(
                    qTs[po : po + 64, 512 * half : 512 * (half + 1)],
                    qT[:, 512 * half : 512 * (half + 1)],
                    pbq[po : po + 64, :],
                    _OP.mult,
                )

        def emit_B(h):
            """L matmuls -> |L|*mask -> Ln -> w@v -> (pair-end) normalize."""
            qc, po = h // 2, 64 * (h % 2)
            kT = sb_qk[po : po + 64, 4 + qc, :]
            qTs = qts_of.pop(h)
            sbw = sb_w0 if h % 2 == 0 else sb_w1
            # L matmuls full-width (masked-out cols are computed then zeroed
            # by the mask — buys uniform tiles and one merged Ln per column
            # half). Diag tiles: DVE |L| via sign-bit clear into f32 scratch,
            # then Pool applies the causal mask (bf16 out). Off-diag tiles:
            # ACT Abs straight to bf16. Ln merged to 2 ACT ops per head.
            for ic in range(2):
                nj = 4 * (ic + 1)
                tw = scr_p.tile([P, 8, 512], _bf16, tag="tw")
                for jb in range(nj):
                    pl = mm_p.tile([P, 512], _f32, tag="mm")
                    nc.tensor.matmul(
                        pl[:],
                        lhsT=kT[:, P * jb : P * jb + P],
                        rhs=qTs[po : po + 64, 512 * ic : 512 * (ic + 1)],
                        start=True,
                        stop=True,
                    )
                    # |L| + f32->bf16 truncation in one 2x-rate DVE op: clear
                    # the sign bit of the high u16 lane of each psum float.
                    nc.vector.tensor_scalar(
                        tw[:, jb, :].bitcast(_u16),
                        _hi16(pl[:]),
                        0x7FFF,
                        None,
                        _OP.bitwise_and,
                    )
                    db = jb - 4 * ic  # diag-block variant, 0..3 when on-diag
                    if 0 <= db <= 3:  # causal mask on the otherwise-idle Pool
                        nc.gpsimd.tensor_tensor(
                            tw[:, jb, :], tw[:, jb, :], mtris[:, db, :], _OP.mult
                        )
                nc.scalar.activation(
                    sbw[:, 0:nj, 512 * ic : 512 * (ic + 1)],
                    tw[:, 0:nj, :],
                    _AF.Ln,
                    bias=1.0,
                )

            # --- w @ [v | 1]: rows 0-63 = out'^T (unnormalized), row 64 = s_i
            for ic in range(2):
                pw = pw_p.tile([65, 512], _f32, tag="pw")
                nj = 4 * (ic + 1)
                for jb in range(nj):
                    nc.tensor.matmul(
                        pw[:],
                        lhsT=sb_v[:, jb, h, :],
                        rhs=sbw[:, jb, 512 * ic : 512 * (ic + 1)],
                        start=(jb == 0),
                        stop=(jb == nj - 1),
                    )
                # gather s row (+eps) into the pair tile; copy out unnormalized
                pp0 = 32 * (h % 2)
                nc.scalar.activation(
                    sall[pp0 : pp0 + 1, qc, 512 * ic : 512 * (ic + 1)],
                    pw[64:65, :],
                    _AF.Copy,
                    bias=EPS,
                )
                nc.vector.tensor_scalar(
                    sb_mg[po : po + 64, qc, 512 * ic : 512 * (ic + 1)].bitcast(_u16),
                    _hi16(pw[0:64, :]),
                    0xFFFF,
                    None,
                    _OP.bitwise_and,
                )
        def emit_normalize(qc):
            """Pair-end normalize, deferred into the NEXT head's block so
            the reciprocal lands near the front of the DVE queue instead of
            behind a dozen |L| ops (its own psum pool keeps it off the pbq
            broadcast ring)."""
            with nc.allow_low_precision(
                reason="bf16 rounding of 1/(s+eps) is ~4e-3 relative"
            ):
                nc.vector.reciprocal(srinv_all[:, qc, :], sall[:, qc, :])
            for ic in range(2):
                pinv = pi_p.tile([P, 512], _f32, tag="pinv")
                nc.tensor.matmul(
                    pinv[:],
                    lhsT=sel33[:],
                    rhs=srinv_all[:, qc, 512 * ic : 512 * (ic + 1)],
                    start=True,
                    stop=True,
                )
                nc.vector.tensor_tensor(
                    sb_mg[:, qc, 512 * ic : 512 * (ic + 1)],
                    sb_mg[:, qc, 512 * ic : 512 * (ic + 1)],
                    pinv[:],
                    _OP.mult,
                )

        # P1 and the Ly/exp aggregate phase interleave per head-pair: the
        # ACT-heavy exp work of all 8 heads hides under P1's matmul burst.
        for p in range(4):
            p1_oc(p)
            p1_oc(8 + p)
            emit_A1(2 * p)
            emit_A1(2 * p + 1)
            p1_oc(4 + p)
        p1_v()

        emit_A2(0)
        pending = None
        for h in range(HEADS_PER_CORE):
            if h + 1 < HEADS_PER_CORE:
                emit_A2(h + 1)
            if pending is not None:
                emit_normalize(pending)
                pending = None
            emit_B(h)
            if h % 2 == 1:
                pending = h // 2
        if pending is not None:
            emit_normalize(pending)

        # ---- P3: project (row-parallel partial), output transposed ----
        with tc.tile_pool(name="obuf", bufs=2) as ob_p:
            for cc in range(8):
                ob = ob_p.tile([P, T], _bf16, tag="ob")
                for tn in range(2):
                    ppj = mm_p.tile([P, 512], _f32, tag="mm")
                    for kc in range(4):
                        nc.tensor.matmul(
                            ppj[:],
                            lhsT=sb_wp[:, kc, cc * P : (cc + 1) * P],
                            rhs=sb_mg[:, kc, tn * 512 : (tn + 1) * 512],
                            start=(kc == 0),
                            stop=(kc == 3),
                        )
                    if tn == 0:
                        nc.vector.tensor_scalar(
                            ob[:, 0:512].bitcast(_u16),
                            _hi16(ppj[:]),
                            0xFFFF,
                            None,
                            _OP.bitwise_and,
                        )
                    else:
                        nc.scalar.copy(ob[:, 512:1024], ppj[:])
                eng = nc.sync if cc % 2 == 0 else nc.scalar
                eng.dma_start(oT_d[cc * P : (cc + 1) * P, :], ob[:])

    _split_waits(nc)
    return nc


_NC_CACHE = None


def _get_nc():
    global _NC_CACHE
    if _NC_CACHE is None:
        _NC_CACHE = build_nc()
    return _NC_CACHE


def shard_inputs(x, w_qkv, w_ky, w_proj):
    """Host-side shard/layout prep. Core c: batch c//2, heads 8*(c%2)..+8."""
    bf = ml_dtypes.bfloat16
    x = np.asarray(x, np.float32)
    w_qkv = np.asarray(w_qkv, np.float32)
    w_ky = np.asarray(w_ky, np.float32)
    w_proj = np.asarray(w_proj, np.float32)

    trilm1 = np.tril(np.ones((P, P), np.float32)) - 1.0
    # mtris[p, v, c] = 1 where key-partition p is causally visible from query
    # column c for a diag-block at column offset 128v: p <= c - 128v.
    mtris = np.zeros((P, 4, 512), bf)
    triu = np.triu(np.ones((P, P), np.float32))
    for v in range(4):
        mtris[:, v, 128 * v : 128 * v + 128] = triu
        mtris[:, v, 128 * v + 128 :] = 1.0
    identb = np.eye(P, dtype=bf)
    e8 = np.zeros((8, 8 * P), bf)
    for cc in range(8):
        e8[cc, cc * P : (cc + 1) * P] = 1.0
    sel33 = np.zeros((33, P), bf)
    sel33[0, 0:64] = 1.0
    sel33[32, 64:128] = 1.0

    in_maps = []
    for c in range(8):
        b, h0 = c // 2, 8 * (c % 2)
        r0 = h0 * 64
        wq = w_qkv[r0 : r0 + 512]
        wk = w_qkv[D + r0 : D + r0 + 512]
        wky = w_ky[r0 : r0 + 512]
        wv = w_qkv[2 * D + r0 : 2 * D + r0 + 512]
        in_maps.append(
            {
                "xT": np.ascontiguousarray(x[b].T).astype(bf),
                "wqkkT": np.ascontiguousarray(
                    np.concatenate([wq, wk, wky], axis=0).T
                ).astype(bf),
                "wvT": np.ascontiguousarray(wv.T).astype(bf),
                "wpT": np.ascontiguousarray(w_proj[:, r0 : r0 + 512].T).astype(bf),
                "identb": identb,
                "trilm1": trilm1,
                "mtris": mtris,
                "e8": e8,
                "sel33": sel33,
            }
        )
    return in_maps


def unshard_output(results):
    """results: list of 8 dicts with 'oT' [D, T] bf16 partials. Sum pairs, transpose."""
    out = np.empty((B, T, D), np.float32)
    for b in range(B):
        acc = results[2 * b]["oT"].astype(np.float32) + results[2 * b + 1][
            "oT"
        ].astype(np.float32)
        out[b] = acc.T
    return out


def kernel(**inputs):
    from concourse.bass_utils import run_bass_kernel_spmd

    nc = _get_nc()
    in_maps = shard_inputs(
        inputs["x"], inputs["w_qkv"], inputs["w_ky"], inputs["w_proj"]
    )
    res = run_bass_kernel_spmd(nc, in_maps, list(range(8)))
    return unshard_output(res.results)


if __name__ == "__main__":
    rng = np.random.default_rng(0)
    ins = {
        "x": rng.normal(size=(B, T, D)).astype(np.float32),
        "w_qkv": rng.normal(size=(3 * D, D)).astype(np.float32) * 0.003,
        "w_ky": rng.normal(size=(D, D)).astype(np.float32) * 0.003,
        "w_proj": rng.normal(size=(D, D)).astype(np.float32) * 0.003,
    }
    out = kernel(**ins)
    print("kernel output", out.shape, out.dtype)


# revision 70
# speedup vs baseline: 1.1807x; 1.0106x over previous
"""Trainium2 Bass kernel for nn_CausalSelfAttention_74938589380902.

Reference computation (B=4, T=1024, D=1024, H=16, hd=64):
    qkv = x @ w_qkv.T ; split heads
    L   = (q k^T)/8 ; L_y = (q k_y^T)/8  (k_y from separate projection)
    agg = sum(exp(clip(L_y)) * tril) + eps              (per query)
    w   = softplus(log(|L|+eps) - log(agg+eps)) * tril  = log1p((|L|+eps)/(agg+eps)) * tril
    A   = w / (sum(w) + eps) ; out = (A v) merged @ w_proj.T

Sharding: 8 cores = 4 batches x 2 head-groups (8 heads each). Each core
computes its batch/head slice end-to-end and a partial (row-parallel)
projection output, transposed; the host sums the pair of partials per batch.

Design (vs the f32r v1 baseline: 462us -> ~313us measured NTFF exec/core;
the kernel is PE-bound at ~95% matmul occupancy, clocks power-throttled to
~1.35GHz in matmul-only phases and ~0.85GHz when all engines run):
  - all matmul operands bf16 (f32r pays 4 cyc/row under 256-row moving dims
    and double DMA; bf16 is 1 cyc/row always)
  - inputs stream as 8 x-chunks + 12 weight tiles over the SP + ACT HWDGE
    rings (first matmul at ~11us instead of 45us; each dma_start costs
    ~1.1-1.3us of ring sequencer time, so chunk count is balanced)
  - P1 (qkv projections) and the Ly/exp aggregate phase interleave per
    head-pair so all 8 heads' ACT exp work hides under P1's matmul burst
  - A2/B per-head pipeline is software-pipelined (A2(h+1) emitted before
    B(h)); within B all L matmuls precede all wv matmuls so Ln/abs of one
    column half digests while the PE runs the other
  - full-width L tiles: masked-out columns are computed then zeroed by the
    causal mask, buying uniform [128,512] tiles and one merged Ln per
    column half (2 ACT Ln ops/head instead of 12)
  - |L| + f32->bf16 conversion fused into one 2x-rate DVE op (bitwise_and
    0x7FFF on the high u16 lane of each psum float); causal masks as plain
    bf16 tensor_tensor on the otherwise-idle Pool engine (Pool cannot
    access PSUM and rejects scalar_tensor_tensor entirely)
  - exp row-aggregates use full-width exps with free-axis accum plus a
    Pool-free (tril-1)-weighted correction op subtracting the masked part
  - row-sum reciprocals batched per head-pair on partitions {0,32} of a
    persistent [33,4,T] tile (engines can only address partition bases at
    multiples of 32); one [33,1024] DVE reciprocal per pair instead of 16
    single-partition [1,512] ops at 2.5us each
  - merged output written unnormalized (u16-lane truncating copy), then
    scaled in place with one [128,512] DVE mult per (pair, ic) via a
    sel33-broadcast matmul; output DMA'd as bf16 and upcast on host
  - log1p trick retained: w = Ln(|L_raw| * (0.125/(agg+2eps)) + 1)
  - row sums of w come free from the w@v matmul (ones column in v, M=65)
Verified numerics: absmax-relative err 1.24e-2 vs the fp32 reference
(gate 2e-2). Known-failed variants: deferred pair-end normalize (+6us),
masks on DVE instead of Pool (+3us), mm psum ring 4->3 bufs (+5us),
fp8 DoubleRow anywhere in the value path (zero-mean dot products keep
the ~5% per-element fp8 noise, blowing the absmax gate).
"""

import sys

sys.path.insert(0, "/opt/trn_rl_repo")

import ml_dtypes
import numpy as np

import concourse.bass as bass
import concourse.mybir as mybir
import concourse.tile as tile
from contextlib import ExitStack

P = 128
T = 1024
D = 1024
B = 4
HEADS_PER_CORE = 8
EPS = 1e-6

_f32 = mybir.dt.float32
_bf16 = mybir.dt.bfloat16
_u16 = mybir.dt.uint16
_u32 = mybir.dt.uint32


def _hi16(ap):
    """High u16 lane of an f32 AP — bf16 payload bits of each float."""
    return ap.bitcast(_u16).rearrange("p (c two) -> p c two", two=2)[:, :, 1]
_AF = mybir.ActivationFunctionType
_OP = mybir.AluOpType
_AX = mybir.AxisListType


def _split_waits(nc, max_waits=1, drain_max=1):
    """Walrus' per-instruction codegen rejects >2 sync-wait commands (the
    Drain CTRL struct rejects >=3; a Matmult S3_LW struct rejected 4). Hoist
    excess waits onto NOPs inserted right before the instruction — the NOP
    blocks the same engine queue, so semantics are preserved."""
    for bb in nc.main_func.blocks:
        idx = 0
        while idx < len(bb.instructions):
            ins = bb.instructions[idx]
            si = ins.sync_info
            if si is None:
                idx += 1
                continue
            limit = drain_max if type(ins).__name__ == "InstDrain" else max_waits
            waits = list(si.on_wait)
            if len(waits) <= limit:
                idx += 1
                continue
            keep, excess = waits[:limit], waits[limit:]
            nops = []
            for i in range(0, len(excess), max_waits):
                nop = mybir.InstNoOp(name=nc.get_next_instruction_name(), ins=[], outs=[])
                nop.engine = ins.engine
                nop.sync_info = mybir.SyncInfo(
                    on_wait=excess[i : i + max_waits], on_update=[]
                )
                nops.append(nop)
            ins.sync_info = mybir.SyncInfo(on_wait=keep, on_update=list(si.on_update))
            for j, nop in enumerate(nops):
                bb.instructions.insert(idx + j, nop)
                nc.register_instruction(nop)
            idx += len(nops) + 1


def build_nc():
    """Build the single-core SPMD program (per-core data arrives as inputs)."""
    nc = bass.Bass()

    xT_d = nc.dram_tensor("xT", [D, T], _bf16, kind="ExternalInput").ap()
    wqk_d = nc.dram_tensor("wqkkT", [D, 1536], _bf16, kind="ExternalInput").ap()
    wvT_d = nc.dram_tensor("wvT", [D, 512], _bf16, kind="ExternalInput").ap()
    wpT_d = nc.dram_tensor("wpT", [512, D], _bf16, kind="ExternalInput").ap()
    identb_d = nc.dram_tensor("identb", [P, P], _bf16, kind="ExternalInput").ap()
    trilm1_d = nc.dram_tensor("trilm1", [P, P], _f32, kind="ExternalInput").ap()
    mtris_d = nc.dram_tensor("mtris", [P, 4, 512], _bf16, kind="ExternalInput").ap()
    e8_d = nc.dram_tensor("e8", [8, 8 * P], _bf16, kind="ExternalInput").ap()
    sel33_d = nc.dram_tensor("sel33", [33, P], _bf16, kind="ExternalInput").ap()
    oT_d = nc.dram_tensor("oT", [D, T], _bf16, kind="ExternalOutput").ap()

    with tile.TileContext(nc) as tc, ExitStack() as ctx:
        # ---- persistent SBUF pools ----
        const_p = ctx.enter_context(tc.tile_pool(name="const", bufs=1))
        qk_p = ctx.enter_context(tc.tile_pool(name="qkky", bufs=1))
        v_p = ctx.enter_context(tc.tile_pool(name="vbuf", bufs=1))
        w_p = ctx.enter_context(tc.tile_pool(name="wbuf", bufs=1))
        mg_p = ctx.enter_context(tc.tile_pool(name="merged", bufs=1))
        x_p = ctx.enter_context(tc.tile_pool(name="xT", bufs=1))
        wv_p = ctx.enter_context(tc.tile_pool(name="wvT", bufs=1))
        wp_p = ctx.enter_context(tc.tile_pool(name="wproj", bufs=1))

        identb = const_p.tile([P, P], _bf16)
        nc.gpsimd.dma_start(identb[:], identb_d[:])
        trilm1 = const_p.tile([P, P], _f32)
        nc.gpsimd.dma_start(trilm1[:], trilm1_d[:])
        mtris = const_p.tile([P, 4, 512], _bf16)
        nc.gpsimd.dma_start(mtris[:], mtris_d[:])
        e8 = const_p.tile([8, 8 * P], _bf16)
        nc.gpsimd.dma_start(e8[:], e8_d[:])
        sel33 = const_p.tile([33, P], _bf16)
        nc.gpsimd.dma_start(sel33[:], sel33_d[:])
        eps16 = const_p.tile([P, 8], _f32)
        nc.gpsimd.memset(eps16[:], 16.0 * EPS)
        # s rows per (parity -> partition 0/32, pair, i); garbage partitions
        # stay 1.0 so the sel33 broadcast matmul sees finite values.
        sall = const_p.tile([33, 4, T], _f32)
        nc.gpsimd.memset(sall[:], 1.0)
        srinv_all = const_p.tile([33, 4, T], _bf16)

        sb_qk = qk_p.tile([P, 12, T], _bf16)  # qT(0-3) kT(4-7) kyT(8-11)
        sb_v = v_p.tile([P, 8, 8, 65], _bf16)  # [t_in, t_blk, head, hd + ones]
        sb_w0 = w_p.tile([P, 8, T], _bf16)  # per-head w (even heads), [j_in, jb, i]
        sb_w1 = w_p.tile([P, 8, T], _bf16)  # per-head w (odd heads)
        sb_mg = mg_p.tile([P, 4, T], _bf16)  # mergedT (A@v; normalized in place)

        # ones column of v via Pool memset (keeps the HWDGE rings for loads);
        # w tiles need no zero-fill — the full-width masked Ln writes every
        # column that the wv matmuls read.
        nc.gpsimd.memset(sb_v[:, :, :, 64], 1.0)

        # ---- streamed input loads (SP + ACT HWDGE rings in parallel) ----
        # Ring order: first wqk tile + x chunks first; late-phase weights
        # (wv, wp) at the back. Each dma_start costs ~1.1us of its ring's
        # sequencer, so chunks are sized to balance latency vs issue count.
        # 8 separate x tiles so each accumulation step waits only its own
        # chunk's DMA (a single tile would gate P1 on the whole 2MB load).
        sb_xc = []
        for dc in range(8):
            xt = x_p.tile([P, T], _bf16, tag=f"xc{dc}")
            sb_xc.append(xt)
        xs = xT_d.rearrange("(dc p) t -> p dc t", p=P)
        ws_p = ctx.enter_context(tc.tile_pool(name="wstream", bufs=1))
        wt_of = {}
        oc_order = [0, 8, 4, 1, 9, 5, 2, 10, 6, 3, 11, 7]
        for i, oc in enumerate(oc_order):
            wt = ws_p.tile([P, 8, P], _bf16, tag=f"wt{oc}")
            wt_of[oc] = wt
        nc.scalar.dma_start(
            wt_of[0][:],
            wqk_d[:, 0:P].rearrange("(dc p) o -> p dc o", p=P),
        )
        for dc in range(8):
            eng = nc.sync if dc % 2 == 0 else nc.scalar
            eng.dma_start(sb_xc[dc][:], xs[:, dc, :])
        for i, oc in enumerate(oc_order[1:]):
            eng = nc.sync if i % 2 == 0 else nc.scalar
            eng.dma_start(
                wt_of[oc][:],
                wqk_d[:, oc * P : (oc + 1) * P].rearrange("(dc p) o -> p dc o", p=P),
            )
        sb_wv = wv_p.tile([P, 8, 512], _bf16)
        nc.sync.dma_start(sb_wv[:], wvT_d.rearrange("(dc p) o -> p dc o", p=P))
        sb_wp = wp_p.tile([P, 4, T], _bf16)  # wpT [i'_in, i'_chunk, c]
        nc.scalar.dma_start(sb_wp[:], wpT_d.rearrange("(kc p) c -> p kc c", p=P))

        # shared psum pools (8 banks: 4 mm + 1 pbq/pinv + 2 pw + 1 ptr)
        mm_p = ctx.enter_context(tc.tile_pool(name="mm_ps", bufs=4, space="PSUM"))
        bc_p = ctx.enter_context(tc.tile_pool(name="bc_ps", bufs=1, space="PSUM"))
        pw_p = ctx.enter_context(tc.tile_pool(name="pw_ps", bufs=2, space="PSUM"))
        ptr_p = ctx.enter_context(tc.tile_pool(name="ptr_ps", bufs=1, space="PSUM"))
        # sbuf scratch pools
        scr_p = ctx.enter_context(tc.tile_pool(name="scr", bufs=2))
        qts_p = ctx.enter_context(tc.tile_pool(name="qts", bufs=3))
        sm_p = ctx.enter_context(tc.tile_pool(name="small", bufs=2))

        def p1_oc(oc):
            """Project one 128-col chunk of q/k/ky into sb_qk (transposed)."""
            wt = wt_of[oc]
            for tn in range(2):
                pt = mm_p.tile([P, 512], _f32, tag="mm")
                for dc in range(8):
                    nc.tensor.matmul(
                        pt[:],
                        lhsT=wt[:, dc, :],
                        rhs=sb_xc[dc][:, tn * 512 : (tn + 1) * 512],
                        start=(dc == 0),
                        stop=(dc == 7),
                    )
                nc.vector.tensor_scalar(
                    sb_qk[:, oc, tn * 512 : (tn + 1) * 512].bitcast(_u16),
                    _hi16(pt[:]),
                    0xFFFF,
                    None,
                    _OP.bitwise_and,
                )

        def p1_v():
            for tb in range(8):
                pt = mm_p.tile([P, 512], _f32, tag="mm")
                for dc in range(8):
                    nc.tensor.matmul(
                        pt[:],
                        lhsT=sb_xc[dc][:, tb * P : (tb + 1) * P],
                        rhs=sb_wv[:, dc, :],
                        start=(dc == 0),
                        stop=(dc == 7),
                    )
                nc.vector.tensor_scalar(
                    sb_v[:, tb, :, 0:64].bitcast(_u16),
                    _hi16(pt[:]).rearrange("p (h e) -> p h e", h=8),
                    0xFFFF,
                    None,
                    _OP.bitwise_and,
                )

        # ---- P2: attention, software-pipelined across heads ----
        aggs_of = {}
        qts_of = {}

        def emit_A1(h):
            """Ly matmuls + exp row-aggregates for head h (PE + ACT + Pool).

            One full-width exp (incl. the unmasked diagonal strip) per
            (b, c) chunk with free-axis accum; the Pool engine then adds
            sum(exp * (tril-1)) over the diagonal strip, subtracting the
            masked-out upper-triangle contribution."""
            qc, po = h // 2, 64 * (h % 2)
            qT = sb_qk[po : po + 64, qc, :]
            kyT = sb_qk[po : po + 64, 8 + qc, :]
            aggs = sm_p.tile([P, 8, 3], _f32, tag="aggs")
            aggs_of[h] = aggs
            nc.gpsimd.memset(aggs[:], 0.0)
            for b in range(8):
                jext = P * b + P
                for c in range(b // 4 + 1):
                    jw = min(512, jext - 512 * c)
                    kw = min(jw, max(0, P * b - 512 * c))
                    ply = mm_p.tile([P, 512], _f32, tag="mm")
                    nc.tensor.matmul(
                        ply[:, :jw],
                        lhsT=qT[:, P * b : P * b + P],
                        rhs=kyT[:, 512 * c : 512 * c + jw],
                        start=True,
                        stop=True,
                    )
                    esc = scr_p.tile([P, 512], _bf16, tag="escr")
                    nc.scalar.activation(
                        esc[:, :jw],
                        ply[:, :jw],
                        _AF.Exp,
                        scale=0.125,
                        accum_out=aggs[:, b, c : c + 1],
                    )
                    if c == b // 4:  # diagonal strip [kw, kw+128)
                        smsk = scr_p.tile([P, P], _bf16, tag="smsk")
                        nc.vector.scalar_tensor_tensor(
                            smsk[:],
                            esc[:, kw : kw + P],
                            1.0,
                            trilm1[:],
                            _OP.mult,
                            _OP.mult,
                            accum_out=aggs[:, b, 2:3],
                        )

        def emit_A2(h):
            """agg -> binv -> broadcast -> scaled q for head h."""
            qc, po = h // 2, 64 * (h % 2)
            qT = sb_qk[po : po + 64, qc, :]
            aggs = aggs_of.pop(h)
            agg8 = sm_p.tile([P, 8], _f32, tag="agg8")
            nc.vector.reduce_sum(agg8[:], aggs[:], axis=_AX.X)
            # b8b = 8*agg + 16eps ; binv = 1/b8b = 0.125/(agg+2eps)
            b8b = sm_p.tile([P, 8], _f32, tag="b8b")
            nc.vector.scalar_tensor_tensor(
                b8b[:], agg8[:], 8.0, eps16[:], _OP.mult, _OP.add
            )
            binv = sm_p.tile([P, 8], _bf16, tag="binv")
            with nc.allow_low_precision(
                reason="bf16 rounding of 1/(8agg+16eps) is ~4e-3 relative"
            ):
                nc.vector.reciprocal(binv[:], b8b[:])
            ptr = ptr_p.tile([8, P], _bf16, tag="ptr")
            nc.tensor.transpose(ptr[:], binv[:], identb[:])
            btr = sm_p.tile([8, P], _bf16, tag="btr")
            nc.vector.tensor_scalar(btr[:], ptr[:], 0.0, None, _OP.add)
            qTs = qts_p.tile([P, T], _bf16, tag="qts")
            qts_of[h] = qTs
            for half in range(2):
                pbq = bc_p.tile([P, 512], _f32, tag="bc")
                for cc in range(4):
                    nc.tensor.matmul(
                        pbq[:, cc * P : (cc + 1) * P],
                        lhsT=e8[:, (4 * half + cc) * P : (4 * half + cc + 1) * P],
                        rhs=btr[:],
                        start=True,
                        stop=True,
                    )
                nc.vector.tensor_tensor(
                    qTs[po : po + 64, 512 * half : 512 * (half + 1)],
                    qT[:, 512 * half : 512 * (half + 1)],
                    pbq[po : po + 64, :],
                    _OP.mult,
                )

        def emit_B(h):
            """L matmuls -> |L|*mask -> Ln -> w@v -> (pair-end) normalize."""
            qc, po = h // 2, 64 * (h % 2)
            kT = sb_qk[po : po + 64, 4 + qc, :]
            qTs = qts_of.pop(h)
            sbw = sb_w0 if h % 2 == 0 else sb_w1
            # L matmuls full-width (masked-out cols are computed then zeroed
            # by the mask — buys uniform tiles and one merged Ln per column
            # half). Diag tiles: DVE |L| via sign-bit clear into f32 scratch,
            # then Pool applies the causal mask (bf16 out). Off-diag tiles:
            # ACT Abs straight to bf16. Ln merged to 2 ACT ops per head.
            for ic in range(2):
                nj = 4 * (ic + 1)
                tw = scr_p.tile([P, 8, 512], _bf16, tag="tw")
                for jb in range(nj):
                    pl = mm_p.tile([P, 512], _f32, tag="mm")
                    nc.tensor.matmul(
                        pl[:],
                        lhsT=kT[:, P * jb : P * jb + P],
                        rhs=qTs[po : po + 64, 512 * ic : 512 * (ic + 1)],
                        start=True,
                        stop=True,
                    )
                    # |L| + f32->bf16 truncation in one 2x-rate DVE op: clear
                    # the sign bit of the high u16 lane of each psum float.
                    nc.vector.tensor_scalar(
                        tw[:, jb, :].bitcast(_u16),
                        _hi16(pl[:]),
                        0x7FFF,
                        None,
                        _OP.bitwise_and,
                    )
                    db = jb - 4 * ic  # diag-block variant, 0..3 when on-diag
                    if 0 <= db <= 3:  # causal mask on the otherwise-idle Pool
                        nc.gpsimd.tensor_tensor(
                            tw[:, jb, :], tw[:, jb, :], mtris[:, db, :], _OP.mult
                        )
                nc.scalar.activation(
                    sbw[:, 0:nj, 512 * ic : 512 * (ic + 1)],
                    tw[:, 0:nj, :],
                    _AF.Ln,
                    bias=1.0,
                )

            # --- w @ [v | 1]: rows 0-63 = out'^T (unnormalized), row 64 = s_i
            for ic in range(2):
                pw = pw_p.tile([65, 512], _f32, tag="pw")
                nj = 4 * (ic + 1)
                for jb in range(nj):
                    nc.tensor.matmul(
                        pw[:],
                        lhsT=sb_v[:, jb, h, :],
                        rhs=sbw[:, jb, 512 * ic : 512 * (ic + 1)],
                        start=(jb == 0),
                        stop=(jb == nj - 1),
                    )
                # gather s row (+eps) into the pair tile; copy out unnormalized
                pp0 = 32 * (h % 2)
                nc.scalar.activation(
                    sall[pp0 : pp0 + 1, qc, 512 * ic : 512 * (ic + 1)],
                    pw[64:65, :],
                    _AF.Copy,
                    bias=EPS,
                )
                nc.vector.tensor_scalar(
                    sb_mg[po : po + 64, qc, 512 * ic : 512 * (ic + 1)].bitcast(_u16),
                    _hi16(pw[0:64, :]),
                    0xFFFF,
                    None,
                    _OP.bitwise_and,
                )
            if h % 2 == 1:  # pair end: normalize both heads' merged in place
                with nc.allow_low_precision(
                    reason="bf16 rounding of 1/(s+eps) is ~4e-3 relative"
                ):
                    nc.vector.reciprocal(srinv_all[:, qc, :], sall[:, qc, :])
                for ic in range(2):
                    pinv = bc_p.tile([P, 512], _f32, tag="bc")
                    nc.tensor.matmul(
                        pinv[:],
                        lhsT=sel33[:],
                        rhs=srinv_all[:, qc, 512 * ic : 512 * (ic + 1)],
                        start=True,
                        stop=True,
                    )
                    nc.vector.tensor_tensor(
                        sb_mg[:, qc, 512 * ic : 512 * (ic + 1)],
                        sb_mg[:, qc, 512 * ic : 512 * (ic + 1)],
                        pinv[:],
                        _OP.mult,
                    )

        # P1 and the Ly/exp aggregate phase interleave per head-pair: the
        # ACT-heavy exp work of all 8 heads hides under P1's matmul burst.
        for p in range(4):
            p1_oc(p)
            p1_oc(8 + p)
            emit_A1(2 * p)
            emit_A1(2 * p + 1)
            p1_oc(4 + p)
        p1_v()

        emit_A2(0)
        for h in range(HEADS_PER_CORE):
            if h + 1 < HEADS_PER_CORE:
                emit_A2(h + 1)
            emit_B(h)

        # ---- P3: project (row-parallel partial), output transposed ----
        with tc.tile_pool(name="obuf", bufs=2) as ob_p:
            for cc in range(8):
                ob = ob_p.tile([P, T], _bf16, tag="ob")
                for tn in range(2):
                    ppj = mm_p.tile([P, 512], _f32, tag="mm")
                    for kc in range(4):
                        nc.tensor.matmul(
                            ppj[:],
                            lhsT=sb_wp[:, kc, cc * P : (cc + 1) * P],
                            rhs=sb_mg[:, kc, tn * 512 : (tn + 1) * 512],
                            start=(kc == 0),
                            stop=(kc == 3),
                        )
                    if tn == 0:
                        nc.vector.tensor_scalar(
                            ob[:, 0:512].bitcast(_u16),
                            _hi16(ppj[:]),
                            0xFFFF,
                            None,
                            _OP.bitwise_and,
                        )
                    else:
                        nc.scalar.copy(ob[:, 512:1024], ppj[:])
                eng = nc.sync if cc % 2 == 0 else nc.scalar
                eng.dma_start(oT_d[cc * P : (cc + 1) * P, :], ob[:])

    _split_waits(nc)
    return nc


_NC_CACHE = None


def _get_nc():
    global _NC_CACHE
    if _NC_CACHE is None:
        _NC_CACHE = build_nc()
    return _NC_CACHE


def shard_inputs(x, w_qkv, w_ky, w_proj):
    """Host-side shard/layout prep. Core c: batch c//2, heads 8*(c%2)..+8."""
    bf = ml_dtypes.bfloat16
    x = np.asarray(x, np.float32)
    w_qkv = np.asarray(w_qkv, np.float32)
    w_ky = np.asarray(w_ky, np.float32)
    w_proj = np.asarray(w_proj, np.float32)

    trilm1 = np.tril(np.ones((P, P), np.float32)) - 1.0
    # mtris[p, v, c] = 1 where key-partition p is causally visible from query
    # column c for a diag-block at column offset 128v: p <= c - 128v.
    mtris = np.zeros((P, 4, 512), bf)
    triu = np.triu(np.ones((P, P), np.float32))
    for v in range(4):
        mtris[:, v, 128 * v : 128 * v + 128] = triu
        mtris[:, v, 128 * v + 128 :] = 1.0
    identb = np.eye(P, dtype=bf)
    e8 = np.zeros((8, 8 * P), bf)
    for cc in range(8):
        e8[cc, cc * P : (cc + 1) * P] = 1.0
    sel33 = np.zeros((33, P), bf)
    sel33[0, 0:64] = 1.0
    sel33[32, 64:128] = 1.0

    in_maps = []
    for c in range(8):
        b, h0 = c // 2, 8 * (c % 2)
        r0 = h0 * 64
        wq = w_qkv[r0 : r0 + 512]
        wk = w_qkv[D + r0 : D + r0 + 512]
        wky = w_ky[r0 : r0 + 512]
        wv = w_qkv[2 * D + r0 : 2 * D + r0 + 512]
        in_maps.append(
            {
                "xT": np.ascontiguousarray(x[b].T).astype(bf),
                "wqkkT": np.ascontiguousarray(
                    np.concatenate([wq, wk, wky], axis=0).T
                ).astype(bf),
                "wvT": np.ascontiguousarray(wv.T).astype(bf),
                "wpT": np.ascontiguousarray(w_proj[:, r0 : r0 + 512].T).astype(bf),
                "identb": identb,
                "trilm1": trilm1,
                "mtris": mtris,
                "e8": e8,
                "sel33": sel33,
            }
        )
    return in_maps


def unshard_output(results):
    """results: list of 8 dicts with 'oT' [D, T] bf16 partials. Sum pairs, transpose."""
    out = np.empty((B, T, D), np.float32)
    for b in range(B):
        acc = results[2 * b]["oT"].astype(np.float32) + results[2 * b + 1][
            "oT"
        ].astype(np.float32)
        out[b] = acc.T
    return out


def kernel(**inputs):
    from concourse.bass_utils import run_bass_kernel_spmd

    nc = _get_nc()
    in_maps = shard_inputs(
        inputs["x"], inputs["w_qkv"], inputs["w_ky"], inputs["w_proj"]
    )
    res = run_bass_kernel_spmd(nc, in_maps, list(range(8)))
    return unshard_output(res.results)


if __name__ == "__main__":
    rng = np.random.default_rng(0)
    ins = {
        "x": rng.normal(size=(B, T, D)).astype(np.float32),
        "w_qkv": rng.normal(size=(3 * D, D)).astype(np.float32) * 0.003,
        "w_ky": rng.normal(size=(D, D)).astype(np.float32) * 0.003,
        "w_proj": rng.normal(size=(D, D)).astype(np.float32) * 0.003,
    }
    out = kernel(**ins)
    print("kernel output", out.shape, out.dtype)


# revision 76
# speedup vs baseline: 1.1912x; 1.0089x over previous
"""Trainium2 Bass kernel for nn_CausalSelfAttention_74938589380902.

Reference computation (B=4, T=1024, D=1024, H=16, hd=64):
    qkv = x @ w_qkv.T ; split heads
    L   = (q k^T)/8 ; L_y = (q k_y^T)/8  (k_y from separate projection)
    agg = sum(exp(clip(L_y)) * tril) + eps              (per query)
    w   = softplus(log(|L|+eps) - log(agg+eps)) * tril  = log1p((|L|+eps)/(agg+eps)) * tril
    A   = w / (sum(w) + eps) ; out = (A v) merged @ w_proj.T

Sharding: 8 cores = 4 batches x 2 head-groups (8 heads each). Each core
computes its batch/head slice end-to-end and a partial (row-parallel)
projection output, transposed; the host sums the pair of partials per batch.

Design (vs the f32r v1 baseline: 462us -> ~313us measured NTFF exec/core;
the kernel is PE-bound at ~95% matmul occupancy, clocks power-throttled to
~1.35GHz in matmul-only phases and ~0.85GHz when all engines run):
  - all matmul operands bf16 (f32r pays 4 cyc/row under 256-row moving dims
    and double DMA; bf16 is 1 cyc/row always)
  - inputs stream as 8 x-chunks + 12 weight tiles over the SP + ACT HWDGE
    rings (first matmul at ~11us instead of 45us; each dma_start costs
    ~1.1-1.3us of ring sequencer time, so chunk count is balanced)
  - P1 (qkv projections) and the Ly/exp aggregate phase interleave per
    head-pair so all 8 heads' ACT exp work hides under P1's matmul burst
  - A2/B per-head pipeline is software-pipelined (A2(h+1) emitted before
    B(h)); within B all L matmuls precede all wv matmuls so Ln/abs of one
    column half digests while the PE runs the other
  - full-width L tiles: masked-out columns are computed then zeroed by the
    causal mask, buying uniform [128,512] tiles and one merged Ln per
    column half (2 ACT Ln ops/head instead of 12)
  - |L| + f32->bf16 conversion fused into one 2x-rate DVE op (bitwise_and
    0x7FFF on the high u16 lane of each psum float); causal masks as plain
    bf16 tensor_tensor on the otherwise-idle Pool engine (Pool cannot
    access PSUM and rejects scalar_tensor_tensor entirely)
  - exp row-aggregates use full-width exps with free-axis accum plus a
    Pool-free (tril-1)-weighted correction op subtracting the masked part
  - row-sum reciprocals batched per head-pair on partitions {0,32} of a
    persistent [33,4,T] tile (engines can only address partition bases at
    multiples of 32); one [33,1024] DVE reciprocal per pair instead of 16
    single-partition [1,512] ops at 2.5us each
  - merged output written unnormalized (u16-lane truncating copy), then
    scaled in place with one [128,512] DVE mult per (pair, ic) via a
    sel33-broadcast matmul; output DMA'd as bf16 and upcast on host
  - log1p trick retained: w = Ln(|L_raw| * (0.125/(agg+2eps)) + 1)
  - row sums of w come free from the w@v matmul (ones column in v, M=65)
Verified numerics: absmax-relative err 1.24e-2 vs the fp32 reference
(gate 2e-2). Known-failed variants: deferred pair-end normalize (+6us),
masks on DVE instead of Pool (+3us), mm psum ring 4->3 bufs (+5us),
fp8 DoubleRow anywhere in the value path (zero-mean dot products keep
the ~5% per-element fp8 noise, blowing the absmax gate).
"""

import sys

sys.path.insert(0, "/opt/trn_rl_repo")

import ml_dtypes
import numpy as np

import concourse.bass as bass
import concourse.mybir as mybir
import concourse.tile as tile
from contextlib import ExitStack

P = 128
T = 1024
D = 1024
B = 4
HEADS_PER_CORE = 8
EPS = 1e-6

_f32 = mybir.dt.float32
_bf16 = mybir.dt.bfloat16
_u16 = mybir.dt.uint16
_u32 = mybir.dt.uint32


def _hi16(ap):
    """High u16 lane of an f32 AP — bf16 payload bits of each float."""
    return ap.bitcast(_u16).rearrange("p (c two) -> p c two", two=2)[:, :, 1]
_AF = mybir.ActivationFunctionType
_OP = mybir.AluOpType
_AX = mybir.AxisListType


def _split_waits(nc, max_waits=1, drain_max=1):
    """Walrus' per-instruction codegen rejects >2 sync-wait commands (the
    Drain CTRL struct rejects >=3; a Matmult S3_LW struct rejected 4). Hoist
    excess waits onto NOPs inserted right before the instruction — the NOP
    blocks the same engine queue, so semantics are preserved."""
    for bb in nc.main_func.blocks:
        idx = 0
        while idx < len(bb.instructions):
            ins = bb.instructions[idx]
            si = ins.sync_info
            if si is None:
                idx += 1
                continue
            limit = drain_max if type(ins).__name__ == "InstDrain" else max_waits
            waits = list(si.on_wait)
            if len(waits) <= limit:
                idx += 1
                continue
            keep, excess = waits[:limit], waits[limit:]
            nops = []
            for i in range(0, len(excess), max_waits):
                nop = mybir.InstNoOp(name=nc.get_next_instruction_name(), ins=[], outs=[])
                nop.engine = ins.engine
                nop.sync_info = mybir.SyncInfo(
                    on_wait=excess[i : i + max_waits], on_update=[]
                )
                nops.append(nop)
            ins.sync_info = mybir.SyncInfo(on_wait=keep, on_update=list(si.on_update))
            for j, nop in enumerate(nops):
                bb.instructions.insert(idx + j, nop)
                nc.register_instruction(nop)
            idx += len(nops) + 1


def build_nc():
    """Build the single-core SPMD program (per-core data arrives as inputs)."""
    nc = bass.Bass()

    xT_d = nc.dram_tensor("xT", [D, T], _bf16, kind="ExternalInput").ap()
    wqk_d = nc.dram_tensor("wqkkT", [D, 1536], _bf16, kind="ExternalInput").ap()
    wvT_d = nc.dram_tensor("wvT", [D, 512], _bf16, kind="ExternalInput").ap()
    wpT_d = nc.dram_tensor("wpT", [512, D], _bf16, kind="ExternalInput").ap()
    trilm1_d = nc.dram_tensor("trilm1", [P, P], _f32, kind="ExternalInput").ap()
    mtris_d = nc.dram_tensor("mtris", [P, 4, 512], _bf16, kind="ExternalInput").ap()
    e8_d = nc.dram_tensor("e8", [8, 8 * P], _bf16, kind="ExternalInput").ap()
    sel33_d = nc.dram_tensor("sel33", [33, P], _bf16, kind="ExternalInput").ap()
    oT_d = nc.dram_tensor("oT", [D, T], _bf16, kind="ExternalOutput").ap()

    with tile.TileContext(nc) as tc, ExitStack() as ctx:
        # ---- persistent SBUF pools ----
        const_p = ctx.enter_context(tc.tile_pool(name="const", bufs=1))
        qk_p = ctx.enter_context(tc.tile_pool(name="qkky", bufs=1))
        v_p = ctx.enter_context(tc.tile_pool(name="vbuf", bufs=1))
        w_p = ctx.enter_context(tc.tile_pool(name="wbuf", bufs=1))
        mg_p = ctx.enter_context(tc.tile_pool(name="merged", bufs=1))
        x_p = ctx.enter_context(tc.tile_pool(name="xT", bufs=1))
        wv_p = ctx.enter_context(tc.tile_pool(name="wvT", bufs=1))
        wp_p = ctx.enter_context(tc.tile_pool(name="wproj", bufs=1))

        trilm1 = const_p.tile([P, P], _f32)
        nc.gpsimd.dma_start(trilm1[:], trilm1_d[:])
        mtris = const_p.tile([P, 4, 512], _bf16)
        nc.gpsimd.dma_start(mtris[:], mtris_d[:])
        e8 = const_p.tile([8, 8 * P], _bf16)
        nc.gpsimd.dma_start(e8[:], e8_d[:])
        sel33 = const_p.tile([33, P], _bf16)
        nc.gpsimd.dma_start(sel33[:], sel33_d[:])
        eps16 = const_p.tile([P, 8], _f32)
        nc.gpsimd.memset(eps16[:], 16.0 * EPS)
        # s rows per (parity -> partition 0/32, pair, i); garbage partitions
        # stay 1.0 so the sel33 broadcast matmul sees finite values.
        sall = const_p.tile([33, 4, T], _f32)
        nc.gpsimd.memset(sall[:], 1.0)
        srinv_all = const_p.tile([33, 4, T], _bf16)

        sb_qk = qk_p.tile([P, 12, T], _bf16)  # qT(0-3) kT(4-7) kyT(8-11)
        sb_v = v_p.tile([P, 8, 8, 65], _bf16)  # [t_in, t_blk, head, hd + ones]
        sb_w0 = w_p.tile([P, 8, T], _bf16)  # per-head w (even heads), [j_in, jb, i]
        sb_w1 = w_p.tile([P, 8, T], _bf16)  # per-head w (odd heads)
        sb_mg = mg_p.tile([P, 4, T], _bf16)  # mergedT (A@v; normalized in place)

        # ones column of v via Pool memset (keeps the HWDGE rings for loads);
        # w tiles need no zero-fill — the full-width masked Ln writes every
        # column that the wv matmuls read.
        nc.gpsimd.memset(sb_v[:, :, :, 64], 1.0)

        # ---- streamed input loads (SP + ACT HWDGE rings in parallel) ----
        # Ring order: first wqk tile + x chunks first; late-phase weights
        # (wv, wp) at the back. Each dma_start costs ~1.1us of its ring's
        # sequencer, so chunks are sized to balance latency vs issue count.
        # 8 separate x tiles so each accumulation step waits only its own
        # chunk's DMA (a single tile would gate P1 on the whole 2MB load).
        sb_xc = []
        for dc in range(8):
            xt = x_p.tile([P, T], _bf16, tag=f"xc{dc}")
            sb_xc.append(xt)
        xs = xT_d.rearrange("(dc p) t -> p dc t", p=P)
        ws_p = ctx.enter_context(tc.tile_pool(name="wstream", bufs=1))
        wt_of = {}
        oc_order = [0, 8, 4, 1, 9, 5, 2, 10, 6, 3, 11, 7]
        for i, oc in enumerate(oc_order):
            wt = ws_p.tile([P, 8, P], _bf16, tag=f"wt{oc}")
            wt_of[oc] = wt
        nc.scalar.dma_start(
            wt_of[0][:],
            wqk_d[:, 0:P].rearrange("(dc p) o -> p dc o", p=P),
        )
        for dc in range(8):
            eng = nc.sync if dc % 2 == 0 else nc.scalar
            eng.dma_start(sb_xc[dc][:], xs[:, dc, :])
        for i, oc in enumerate(oc_order[1:]):
            eng = nc.sync if i % 2 == 0 else nc.scalar
            eng.dma_start(
                wt_of[oc][:],
                wqk_d[:, oc * P : (oc + 1) * P].rearrange("(dc p) o -> p dc o", p=P),
            )
        sb_wv = wv_p.tile([P, 8, 512], _bf16)
        nc.sync.dma_start(sb_wv[:], wvT_d.rearrange("(dc p) o -> p dc o", p=P))
        sb_wp = wp_p.tile([P, 4, T], _bf16)  # wpT [i'_in, i'_chunk, c]
        nc.scalar.dma_start(sb_wp[:], wpT_d.rearrange("(kc p) c -> p kc c", p=P))

        # shared psum pools (8 banks: 4 mm + 1 pbq + 1 pinv + 2 pw); the
        # binv transpose moved to the DMA XBAR, freeing its former bank.
        mm_p = ctx.enter_context(tc.tile_pool(name="mm_ps", bufs=4, space="PSUM"))
        bc_p = ctx.enter_context(tc.tile_pool(name="bc_ps", bufs=1, space="PSUM"))
        pw_p = ctx.enter_context(tc.tile_pool(name="pw_ps", bufs=2, space="PSUM"))
        pi_p = ctx.enter_context(tc.tile_pool(name="pi_ps", bufs=1, space="PSUM"))
        # sbuf scratch pools
        scr_p = ctx.enter_context(tc.tile_pool(name="scr", bufs=2))
        qts_p = ctx.enter_context(tc.tile_pool(name="qts", bufs=3))
        sm_p = ctx.enter_context(tc.tile_pool(name="small", bufs=2))

        def p1_oc(oc):
            """Project one 128-col chunk of q/k/ky into sb_qk (transposed)."""
            wt = wt_of[oc]
            for tn in range(2):
                pt = mm_p.tile([P, 512], _f32, tag="mm")
                for dc in range(8):
                    nc.tensor.matmul(
                        pt[:],
                        lhsT=wt[:, dc, :],
                        rhs=sb_xc[dc][:, tn * 512 : (tn + 1) * 512],
                        start=(dc == 0),
                        stop=(dc == 7),
                    )
                nc.vector.tensor_scalar(
                    sb_qk[:, oc, tn * 512 : (tn + 1) * 512].bitcast(_u16),
                    _hi16(pt[:]),
                    0xFFFF,
                    None,
                    _OP.bitwise_and,
                )

        def p1_v():
            for tb in range(8):
                pt = mm_p.tile([P, 512], _f32, tag="mm")
                for dc in range(8):
                    nc.tensor.matmul(
                        pt[:],
                        lhsT=sb_xc[dc][:, tb * P : (tb + 1) * P],
                        rhs=sb_wv[:, dc, :],
                        start=(dc == 0),
                        stop=(dc == 7),
                    )
                nc.vector.tensor_scalar(
                    sb_v[:, tb, :, 0:64].bitcast(_u16),
                    _hi16(pt[:]).rearrange("p (h e) -> p h e", h=8),
                    0xFFFF,
                    None,
                    _OP.bitwise_and,
                )

        # ---- P2: attention, software-pipelined across heads ----
        aggs_of = {}
        qts_of = {}

        def emit_A1(h):
            """Ly matmuls + exp row-aggregates for head h (PE + ACT + Pool).

            One full-width exp (incl. the unmasked diagonal strip) per
            (b, c) chunk with free-axis accum; the Pool engine then adds
            sum(exp * (tril-1)) over the diagonal strip, subtracting the
            masked-out upper-triangle contribution."""
            qc, po = h // 2, 64 * (h % 2)
            qT = sb_qk[po : po + 64, qc, :]
            kyT = sb_qk[po : po + 64, 8 + qc, :]
            aggs = sm_p.tile([P, 8, 3], _f32, tag="aggs")
            aggs_of[h] = aggs
            nc.gpsimd.memset(aggs[:], 0.0)
            for b in range(8):
                jext = P * b + P
                for c in range(b // 4 + 1):
                    jw = min(512, jext - 512 * c)
                    kw = min(jw, max(0, P * b - 512 * c))
                    ply = mm_p.tile([P, 512], _f32, tag="mm")
                    nc.tensor.matmul(
                        ply[:, :jw],
                        lhsT=qT[:, P * b : P * b + P],
                        rhs=kyT[:, 512 * c : 512 * c + jw],
                        start=True,
                        stop=True,
                    )
                    esc = scr_p.tile([P, 512], _bf16, tag="escr")
                    nc.scalar.activation(
                        esc[:, :jw],
                        ply[:, :jw],
                        _AF.Exp,
                        scale=0.125,
                        accum_out=aggs[:, b, c : c + 1],
                    )
                    if c == b // 4:  # diagonal strip [kw, kw+128)
                        smsk = scr_p.tile([P, P], _bf16, tag="smsk")
                        nc.vector.scalar_tensor_tensor(
                            smsk[:],
                            esc[:, kw : kw + P],
                            1.0,
                            trilm1[:],
                            _OP.mult,
                            _OP.mult,
                            accum_out=aggs[:, b, 2:3],
                        )

        def emit_A2(h):
            """agg -> binv -> broadcast -> scaled q for head h."""
            qc, po = h // 2, 64 * (h % 2)
            qT = sb_qk[po : po + 64, qc, :]
            aggs = aggs_of.pop(h)
            agg8 = sm_p.tile([P, 8], _f32, tag="agg8")
            nc.vector.reduce_sum(agg8[:], aggs[:], axis=_AX.X)
            # b8b = 8*agg + 16eps ; binv = 1/b8b = 0.125/(agg+2eps)
            b8b = sm_p.tile([P, 8], _f32, tag="b8b")
            nc.vector.scalar_tensor_tensor(
                b8b[:], agg8[:], 8.0, eps16[:], _OP.mult, _OP.add
            )
            # binv padded to [128,128] so the XBAR DMA transpose is legal
            # (free dim must be a multiple of 128, 16-bit dtype): replaces
            # the PE transpose + psum bank + copy. Rows 8-127 of btr are
            # garbage and never read (the e8 broadcast contracts K=8).
            binv = sm_p.tile([P, P], _bf16, tag="binv")
            with nc.allow_low_precision(
                reason="bf16 rounding of 1/(8agg+16eps) is ~4e-3 relative"
            ):
                nc.vector.reciprocal(binv[:, 0:8], b8b[:])
            btr = sm_p.tile([P, P], _bf16, tag="btr")
            eng = nc.sync if h % 2 == 0 else nc.scalar
            eng.dma_start(btr[:], binv[:], transpose=True)
            qTs = qts_p.tile([P, T], _bf16, tag="qts")
            qts_of[h] = qTs
            for half in range(2):
                pbq = bc_p.tile([P, 512], _f32, tag="bc")
                for cc in range(4):
                    nc.tensor.matmul(
                        pbq[:, cc * P : (cc + 1) * P],
                        lhsT=e8[:, (4 * half + cc) * P : (4 * half + cc + 1) * P],
                        rhs=btr[0:8, :],
                        start=True,
                        stop=True,
                    )
                nc.vector.tensor_tensor(
                    qTs[po : po + 64, 512 * half : 512 * (half + 1)],
                    qT[:, 512 * half : 512 * (half + 1)],
                    pbq[po : po + 64, :],
                    _OP.mult,
                )

        def emit_B(h):
            """L matmuls -> |L|*mask -> Ln -> w@v -> (pair-end) normalize."""
            qc, po = h // 2, 64 * (h % 2)
            kT = sb_qk[po : po + 64, 4 + qc, :]
            qTs = qts_of.pop(h)
            sbw = sb_w0 if h % 2 == 0 else sb_w1
            # L matmuls full-width (masked-out cols are computed then zeroed
            # by the mask — buys uniform tiles and one merged Ln per column
            # half). Diag tiles: DVE |L| via sign-bit clear into f32 scratch,
            # then Pool applies the causal mask (bf16 out). Off-diag tiles:
            # ACT Abs straight to bf16. Ln merged to 2 ACT ops per head.
            for ic in range(2):
                nj = 4 * (ic + 1)
                tw = scr_p.tile([P, 8, 512], _bf16, tag="tw")
                for jb in range(nj):
                    pl = mm_p.tile([P, 512], _f32, tag="mm")
                    nc.tensor.matmul(
                        pl[:],
                        lhsT=kT[:, P * jb : P * jb + P],
                        rhs=qTs[po : po + 64, 512 * ic : 512 * (ic + 1)],
                        start=True,
                        stop=True,
                    )
                    # |L| + f32->bf16 truncation in one 2x-rate DVE op: clear
                    # the sign bit of the high u16 lane of each psum float.
                    nc.vector.tensor_scalar(
                        tw[:, jb, :].bitcast(_u16),
                        _hi16(pl[:]),
                        0x7FFF,
                        None,
                        _OP.bitwise_and,
                    )
                    db = jb - 4 * ic  # diag-block variant, 0..3 when on-diag
                    if 0 <= db <= 3:  # causal mask on the otherwise-idle Pool
                        nc.gpsimd.tensor_tensor(
                            tw[:, jb, :], tw[:, jb, :], mtris[:, db, :], _OP.mult
                        )
                nc.scalar.activation(
                    sbw[:, 0:nj, 512 * ic : 512 * (ic + 1)],
                    tw[:, 0:nj, :],
                    _AF.Ln,
                    bias=1.0,
                )

            # --- w @ [v | 1]: rows 0-63 = out'^T (unnormalized), row 64 = s_i
            for ic in range(2):
                pw = pw_p.tile([65, 512], _f32, tag="pw")
                nj = 4 * (ic + 1)
                for jb in range(nj):
                    nc.tensor.matmul(
                        pw[:],
                        lhsT=sb_v[:, jb, h, :],
                        rhs=sbw[:, jb, 512 * ic : 512 * (ic + 1)],
                        start=(jb == 0),
                        stop=(jb == nj - 1),
                    )
                # gather s row (+eps) into the pair tile; copy out unnormalized
                pp0 = 32 * (h % 2)
                nc.scalar.activation(
                    sall[pp0 : pp0 + 1, qc, 512 * ic : 512 * (ic + 1)],
                    pw[64:65, :],
                    _AF.Copy,
                    bias=EPS,
                )
                nc.vector.tensor_scalar(
                    sb_mg[po : po + 64, qc, 512 * ic : 512 * (ic + 1)].bitcast(_u16),
                    _hi16(pw[0:64, :]),
                    0xFFFF,
                    None,
                    _OP.bitwise_and,
                )
        def emit_normalize(qc):
            """Pair-end normalize, deferred into the NEXT head's block so the
            reciprocal lands near the front of the DVE queue instead of
            behind a dozen |L| ops; its own psum pool keeps it off both the
            matmul ring and the pbq broadcast ring."""
            with nc.allow_low_precision(
                reason="bf16 rounding of 1/(s+eps) is ~4e-3 relative"
            ):
                nc.vector.reciprocal(srinv_all[:, qc, :], sall[:, qc, :])
            for ic in range(2):
                pinv = pi_p.tile([P, 512], _f32, tag="pinv")
                nc.tensor.matmul(
                    pinv[:],
                    lhsT=sel33[:],
                    rhs=srinv_all[:, qc, 512 * ic : 512 * (ic + 1)],
                    start=True,
                    stop=True,
                )
                nc.vector.tensor_tensor(
                    sb_mg[:, qc, 512 * ic : 512 * (ic + 1)],
                    sb_mg[:, qc, 512 * ic : 512 * (ic + 1)],
                    pinv[:],
                    _OP.mult,
                )

        # P1 and the Ly/exp aggregate phase interleave per head-pair: the
        # ACT-heavy exp work of all 8 heads hides under P1's matmul burst.
        for p in range(4):
            p1_oc(p)
            p1_oc(8 + p)
            emit_A1(2 * p)
            emit_A1(2 * p + 1)
            p1_oc(4 + p)
        p1_v()

        emit_A2(0)
        pending = None
        for h in range(HEADS_PER_CORE):
            if h + 1 < HEADS_PER_CORE:
                emit_A2(h + 1)
            if pending is not None:
                emit_normalize(pending)
                pending = None
            emit_B(h)
            if h % 2 == 1:
                pending = h // 2
        if pending is not None:
            emit_normalize(pending)

        # ---- P3: project (row-parallel partial), output transposed ----
        with tc.tile_pool(name="obuf", bufs=2) as ob_p:
            for cc in range(8):
                ob = ob_p.tile([P, T], _bf16, tag="ob")
                for tn in range(2):
                    ppj = mm_p.tile([P, 512], _f32, tag="mm")
                    for kc in range(4):
                        nc.tensor.matmul(
                            ppj[:],
                            lhsT=sb_wp[:, kc, cc * P : (cc + 1) * P],
                            rhs=sb_mg[:, kc, tn * 512 : (tn + 1) * 512],
                            start=(kc == 0),
                            stop=(kc == 3),
                        )
                    if tn == 0:
                        nc.vector.tensor_scalar(
                            ob[:, 0:512].bitcast(_u16),
                            _hi16(ppj[:]),
                            0xFFFF,
                            None,
                            _OP.bitwise_and,
                        )
                    else:
                        nc.scalar.copy(ob[:, 512:1024], ppj[:])
                eng = nc.sync if cc % 2 == 0 else nc.scalar
                eng.dma_start(oT_d[cc * P : (cc + 1) * P, :], ob[:])

    _split_waits(nc)
    return nc


_NC_CACHE = None


def _get_nc():
    global _NC_CACHE
    if _NC_CACHE is None:
        _NC_CACHE = build_nc()
    return _NC_CACHE


def shard_inputs(x, w_qkv, w_ky, w_proj):
    """Host-side shard/layout prep. Core c: batch c//2, heads 8*(c%2)..+8."""
    bf = ml_dtypes.bfloat16
    x = np.asarray(x, np.float32)
    w_qkv = np.asarray(w_qkv, np.float32)
    w_ky = np.asarray(w_ky, np.float32)
    w_proj = np.asarray(w_proj, np.float32)

    trilm1 = np.tril(np.ones((P, P), np.float32)) - 1.0
    # mtris[p, v, c] = 1 where key-partition p is causally visible from query
    # column c for a diag-block at column offset 128v: p <= c - 128v.
    mtris = np.zeros((P, 4, 512), bf)
    triu = np.triu(np.ones((P, P), np.float32))
    for v in range(4):
        mtris[:, v, 128 * v : 128 * v + 128] = triu
        mtris[:, v, 128 * v + 128 :] = 1.0
    e8 = np.zeros((8, 8 * P), bf)
    for cc in range(8):
        e8[cc, cc * P : (cc + 1) * P] = 1.0
    sel33 = np.zeros((33, P), bf)
    sel33[0, 0:64] = 1.0
    sel33[32, 64:128] = 1.0

    in_maps = []
    for c in range(8):
        b, h0 = c // 2, 8 * (c % 2)
        r0 = h0 * 64
        wq = w_qkv[r0 : r0 + 512]
        wk = w_qkv[D + r0 : D + r0 + 512]
        wky = w_ky[r0 : r0 + 512]
        wv = w_qkv[2 * D + r0 : 2 * D + r0 + 512]
        in_maps.append(
            {
                "xT": np.ascontiguousarray(x[b].T).astype(bf),
                "wqkkT": np.ascontiguousarray(
                    np.concatenate([wq, wk, wky], axis=0).T
                ).astype(bf),
                "wvT": np.ascontiguousarray(wv.T).astype(bf),
                "wpT": np.ascontiguousarray(w_proj[:, r0 : r0 + 512].T).astype(bf),
                "trilm1": trilm1,
                "mtris": mtris,
                "e8": e8,
                "sel33": sel33,
            }
        )
    return in_maps


def unshard_output(results):
    """results: list of 8 dicts with 'oT' [D, T] bf16 partials. Sum pairs, transpose."""
    out = np.empty((B, T, D), np.float32)
    for b in range(B):
        acc = results[2 * b]["oT"].astype(np.float32) + results[2 * b + 1][
            "oT"
        ].astype(np.float32)
        out[b] = acc.T
    return out


def kernel(**inputs):
    from concourse.bass_utils import run_bass_kernel_spmd

    nc = _get_nc()
    in_maps = shard_inputs(
        inputs["x"], inputs["w_qkv"], inputs["w_ky"], inputs["w_proj"]
    )
    res = run_bass_kernel_spmd(nc, in_maps, list(range(8)))
    return unshard_output(res.results)


if __name__ == "__main__":
    rng = np.random.default_rng(0)
    ins = {
        "x": rng.normal(size=(B, T, D)).astype(np.float32),
        "w_qkv": rng.normal(size=(3 * D, D)).astype(np.float32) * 0.003,
        "w_ky": rng.normal(size=(D, D)).astype(np.float32) * 0.003,
        "w_proj": rng.normal(size=(D, D)).astype(np.float32) * 0.003,
    }
    out = kernel(**ins)
    print("kernel output", out.shape, out.dtype)


# revision 78
# speedup vs baseline: 1.2076x; 1.0138x over previous
"""Trainium2 Bass kernel for nn_CausalSelfAttention_74938589380902.

Reference computation (B=4, T=1024, D=1024, H=16, hd=64):
    qkv = x @ w_qkv.T ; split heads
    L   = (q k^T)/8 ; L_y = (q k_y^T)/8  (k_y from separate projection)
    agg = sum(exp(clip(L_y)) * tril) + eps              (per query)
    w   = softplus(log(|L|+eps) - log(agg+eps)) * tril  = log1p((|L|+eps)/(agg+eps)) * tril
    A   = w / (sum(w) + eps) ; out = (A v) merged @ w_proj.T

Sharding: 8 cores = 4 batches x 2 head-groups (8 heads each). Each core
computes its batch/head slice end-to-end and a partial (row-parallel)
projection output, transposed; the host sums the pair of partials per batch.

Design (vs the f32r v1 baseline: 462us -> ~313us measured NTFF exec/core;
the kernel is PE-bound at ~95% matmul occupancy, clocks power-throttled to
~1.35GHz in matmul-only phases and ~0.85GHz when all engines run):
  - all matmul operands bf16 (f32r pays 4 cyc/row under 256-row moving dims
    and double DMA; bf16 is 1 cyc/row always)
  - inputs stream as 8 x-chunks + 12 weight tiles over the SP + ACT HWDGE
    rings (first matmul at ~11us instead of 45us; each dma_start costs
    ~1.1-1.3us of ring sequencer time, so chunk count is balanced)
  - P1 (qkv projections) and the Ly/exp aggregate phase interleave per
    head-pair so all 8 heads' ACT exp work hides under P1's matmul burst
  - A2/B per-head pipeline is software-pipelined (A2(h+1) emitted before
    B(h)); within B all L matmuls precede all wv matmuls so Ln/abs of one
    column half digests while the PE runs the other
  - full-width L tiles: masked-out columns are computed then zeroed by the
    causal mask, buying uniform [128,512] tiles and one merged Ln per
    column half (2 ACT Ln ops/head instead of 12)
  - |L| + f32->bf16 conversion fused into one 2x-rate DVE op (bitwise_and
    0x7FFF on the high u16 lane of each psum float); causal masks as plain
    bf16 tensor_tensor on the otherwise-idle Pool engine (Pool cannot
    access PSUM and rejects scalar_tensor_tensor entirely)
  - exp row-aggregates use full-width exps with free-axis accum plus a
    Pool-free (tril-1)-weighted correction op subtracting the masked part
  - row-sum reciprocals batched per head-pair on partitions {0,32} of a
    persistent [33,4,T] tile (engines can only address partition bases at
    multiples of 32); one [33,1024] DVE reciprocal per pair instead of 16
    single-partition [1,512] ops at 2.5us each
  - merged output written unnormalized (u16-lane truncating copy), then
    scaled in place with one [128,512] DVE mult per (pair, ic) via a
    sel33-broadcast matmul; output DMA'd as bf16 and upcast on host
  - log1p trick retained: w = Ln(|L_raw| * (0.125/(agg+2eps)) + 1)
  - row sums of w come free from the w@v matmul (ones column in v, M=65)
Verified numerics: absmax-relative err 1.24e-2 vs the fp32 reference
(gate 2e-2). Known-failed variants: deferred pair-end normalize (+6us),
masks on DVE instead of Pool (+3us), mm psum ring 4->3 bufs (+5us),
fp8 DoubleRow anywhere in the value path (zero-mean dot products keep
the ~5% per-element fp8 noise, blowing the absmax gate).
"""

import sys

sys.path.insert(0, "/opt/trn_rl_repo")

import ml_dtypes
import numpy as np

import concourse.bass as bass
import concourse.mybir as mybir
import concourse.tile as tile
from contextlib import ExitStack

P = 128
T = 1024
D = 1024
B = 4
HEADS_PER_CORE = 8
EPS = 1e-6

_f32 = mybir.dt.float32
_bf16 = mybir.dt.bfloat16
_u16 = mybir.dt.uint16
_u32 = mybir.dt.uint32


def _hi16(ap):
    """High u16 lane of an f32 AP — bf16 payload bits of each float."""
    return ap.bitcast(_u16).rearrange("p (c two) -> p c two", two=2)[:, :, 1]
_AF = mybir.ActivationFunctionType
_OP = mybir.AluOpType
_AX = mybir.AxisListType


def _split_waits(nc, max_waits=1, drain_max=1):
    """Walrus' per-instruction codegen rejects >2 sync-wait commands (the
    Drain CTRL struct rejects >=3; a Matmult S3_LW struct rejected 4). Hoist
    excess waits onto NOPs inserted right before the instruction — the NOP
    blocks the same engine queue, so semantics are preserved."""
    for bb in nc.main_func.blocks:
        idx = 0
        while idx < len(bb.instructions):
            ins = bb.instructions[idx]
            si = ins.sync_info
            if si is None:
                idx += 1
                continue
            limit = drain_max if type(ins).__name__ == "InstDrain" else max_waits
            waits = list(si.on_wait)
            if len(waits) <= limit:
                idx += 1
                continue
            keep, excess = waits[:limit], waits[limit:]
            nops = []
            for i in range(0, len(excess), max_waits):
                nop = mybir.InstNoOp(name=nc.get_next_instruction_name(), ins=[], outs=[])
                nop.engine = ins.engine
                nop.sync_info = mybir.SyncInfo(
                    on_wait=excess[i : i + max_waits], on_update=[]
                )
                nops.append(nop)
            ins.sync_info = mybir.SyncInfo(on_wait=keep, on_update=list(si.on_update))
            for j, nop in enumerate(nops):
                bb.instructions.insert(idx + j, nop)
                nc.register_instruction(nop)
            idx += len(nops) + 1


def build_nc():
    """Build the single-core SPMD program (per-core data arrives as inputs)."""
    nc = bass.Bass()

    xT_d = nc.dram_tensor("xT", [D, T], _bf16, kind="ExternalInput").ap()
    wqk_d = nc.dram_tensor("wqkkT", [D, 1536], _bf16, kind="ExternalInput").ap()
    wvT_d = nc.dram_tensor("wvT", [D, 512], _bf16, kind="ExternalInput").ap()
    wpT_d = nc.dram_tensor("wpT", [512, D], _bf16, kind="ExternalInput").ap()
    trilm1_d = nc.dram_tensor("trilm1", [P, P], _f32, kind="ExternalInput").ap()
    mtris_d = nc.dram_tensor("mtris", [P, 4, 512], _bf16, kind="ExternalInput").ap()
    e8_d = nc.dram_tensor("e8", [8, 8 * P], _bf16, kind="ExternalInput").ap()
    sel33_d = nc.dram_tensor("sel33", [33, P], _bf16, kind="ExternalInput").ap()
    oT_d = nc.dram_tensor("oT", [D, T], _bf16, kind="ExternalOutput").ap()

    with tile.TileContext(nc) as tc, ExitStack() as ctx:
        # ---- persistent SBUF pools ----
        const_p = ctx.enter_context(tc.tile_pool(name="const", bufs=1))
        qk_p = ctx.enter_context(tc.tile_pool(name="qkky", bufs=1))
        v_p = ctx.enter_context(tc.tile_pool(name="vbuf", bufs=1))
        w_p = ctx.enter_context(tc.tile_pool(name="wbuf", bufs=1))
        mg_p = ctx.enter_context(tc.tile_pool(name="merged", bufs=1))
        x_p = ctx.enter_context(tc.tile_pool(name="xT", bufs=1))
        wv_p = ctx.enter_context(tc.tile_pool(name="wvT", bufs=1))
        wp_p = ctx.enter_context(tc.tile_pool(name="wproj", bufs=1))

        trilm1 = const_p.tile([P, P], _f32)
        nc.gpsimd.dma_start(trilm1[:], trilm1_d[:])
        mtris = const_p.tile([P, 4, 512], _bf16)
        nc.gpsimd.dma_start(mtris[:], mtris_d[:])
        e8 = const_p.tile([8, 8 * P], _bf16)
        nc.gpsimd.dma_start(e8[:], e8_d[:])
        sel33 = const_p.tile([33, P], _bf16)
        nc.gpsimd.dma_start(sel33[:], sel33_d[:])
        eps16 = const_p.tile([P, 8], _f32)
        nc.gpsimd.memset(eps16[:], 16.0 * EPS)
        # s rows per (parity -> partition 0/32, pair, i); garbage partitions
        # stay 1.0 so the sel33 broadcast matmul sees finite values.
        sall = const_p.tile([33, 4, T], _f32)
        nc.gpsimd.memset(sall[:], 1.0)
        srinv_all = const_p.tile([33, 4, T], _bf16)

        sb_qk = qk_p.tile([P, 12, T], _bf16)  # qT(0-3) kT(4-7) kyT(8-11)
        sb_v = v_p.tile([P, 8, 8, 65], _bf16)  # [t_in, t_blk, head, hd + ones]
        sb_w0 = w_p.tile([P, 8, T], _bf16)  # per-head w (even heads), [j_in, jb, i]
        sb_w1 = w_p.tile([P, 8, T], _bf16)  # per-head w (odd heads)
        sb_mg = mg_p.tile([P, 4, T], _bf16)  # mergedT (A@v; normalized in place)

        # ones column of v via Pool memset (keeps the HWDGE rings for loads);
        # w tiles need no zero-fill — the full-width masked Ln writes every
        # column that the wv matmuls read.
        nc.gpsimd.memset(sb_v[:, :, :, 64], 1.0)

        # ---- streamed input loads (SP + ACT HWDGE rings in parallel) ----
        # Ring order: first wqk tile + x chunks first; late-phase weights
        # (wv, wp) at the back. Each dma_start costs ~1.1us of its ring's
        # sequencer, so chunks are sized to balance latency vs issue count.
        # 8 separate x tiles so each accumulation step waits only its own
        # chunk's DMA (a single tile would gate P1 on the whole 2MB load).
        sb_xc = []
        for dc in range(8):
            xt = x_p.tile([P, T], _bf16, tag=f"xc{dc}")
            sb_xc.append(xt)
        xs = xT_d.rearrange("(dc p) t -> p dc t", p=P)
        ws_p = ctx.enter_context(tc.tile_pool(name="wstream", bufs=1))
        wt_of = {}
        oc_order = [0, 8, 4, 1, 9, 5, 2, 10, 6, 3, 11, 7]
        for i, oc in enumerate(oc_order):
            wt = ws_p.tile([P, 8, P], _bf16, tag=f"wt{oc}")
            wt_of[oc] = wt
        nc.scalar.dma_start(
            wt_of[0][:],
            wqk_d[:, 0:P].rearrange("(dc p) o -> p dc o", p=P),
        )
        for dc in range(8):
            eng = nc.sync if dc % 2 == 0 else nc.scalar
            eng.dma_start(sb_xc[dc][:], xs[:, dc, :])
        for i, oc in enumerate(oc_order[1:]):
            eng = nc.sync if i % 2 == 0 else nc.scalar
            eng.dma_start(
                wt_of[oc][:],
                wqk_d[:, oc * P : (oc + 1) * P].rearrange("(dc p) o -> p dc o", p=P),
            )
        sb_wv = wv_p.tile([P, 8, 512], _bf16)
        nc.sync.dma_start(sb_wv[:], wvT_d.rearrange("(dc p) o -> p dc o", p=P))
        sb_wp = wp_p.tile([P, 4, T], _bf16)  # wpT [i'_in, i'_chunk, c]
        nc.scalar.dma_start(sb_wp[:], wpT_d.rearrange("(kc p) c -> p kc c", p=P))

        # shared psum pools (8 banks: 4 mm + 1 pbq + 1 pinv + 2 pw); the
        # binv transpose moved to the DMA XBAR, freeing its former bank.
        mm_p = ctx.enter_context(tc.tile_pool(name="mm_ps", bufs=4, space="PSUM"))
        bc_p = ctx.enter_context(tc.tile_pool(name="bc_ps", bufs=1, space="PSUM"))
        pw_p = ctx.enter_context(tc.tile_pool(name="pw_ps", bufs=2, space="PSUM"))
        pi_p = ctx.enter_context(tc.tile_pool(name="pi_ps", bufs=1, space="PSUM"))
        # sbuf scratch pools
        scr_p = ctx.enter_context(tc.tile_pool(name="scr", bufs=2))
        qts_p = ctx.enter_context(tc.tile_pool(name="qts", bufs=3))
        sm_p = ctx.enter_context(tc.tile_pool(name="small", bufs=2))

        def p1_oc(oc):
            """Project one 128-col chunk of q/k/ky into sb_qk (transposed)."""
            wt = wt_of[oc]
            for tn in range(2):
                pt = mm_p.tile([P, 512], _f32, tag="mm")
                for dc in range(8):
                    nc.tensor.matmul(
                        pt[:],
                        lhsT=wt[:, dc, :],
                        rhs=sb_xc[dc][:, tn * 512 : (tn + 1) * 512],
                        start=(dc == 0),
                        stop=(dc == 7),
                    )
                nc.vector.tensor_scalar(
                    sb_qk[:, oc, tn * 512 : (tn + 1) * 512].bitcast(_u16),
                    _hi16(pt[:]),
                    0xFFFF,
                    None,
                    _OP.bitwise_and,
                )

        def p1_v():
            for tb in range(8):
                pt = mm_p.tile([P, 512], _f32, tag="mm")
                for dc in range(8):
                    nc.tensor.matmul(
                        pt[:],
                        lhsT=sb_xc[dc][:, tb * P : (tb + 1) * P],
                        rhs=sb_wv[:, dc, :],
                        start=(dc == 0),
                        stop=(dc == 7),
                    )
                nc.vector.tensor_scalar(
                    sb_v[:, tb, :, 0:64].bitcast(_u16),
                    _hi16(pt[:]).rearrange("p (h e) -> p h e", h=8),
                    0xFFFF,
                    None,
                    _OP.bitwise_and,
                )

        # ---- P2: attention, software-pipelined across heads ----
        aggs_of = {}
        qts_of = {}

        def emit_A1(h):
            """Ly matmuls + exp row-aggregates for head h (PE + ACT + Pool).

            One full-width exp (incl. the unmasked diagonal strip) per
            (b, c) chunk with free-axis accum; the Pool engine then adds
            sum(exp * (tril-1)) over the diagonal strip, subtracting the
            masked-out upper-triangle contribution."""
            qc, po = h // 2, 64 * (h % 2)
            qT = sb_qk[po : po + 64, qc, :]
            kyT = sb_qk[po : po + 64, 8 + qc, :]
            aggs = sm_p.tile([P, 8, 3], _f32, tag="aggs")
            aggs_of[h] = aggs
            nc.gpsimd.memset(aggs[:], 0.0)
            for b in range(8):
                jext = P * b + P
                for c in range(b // 4 + 1):
                    jw = min(512, jext - 512 * c)
                    kw = min(jw, max(0, P * b - 512 * c))
                    ply = mm_p.tile([P, 512], _f32, tag="mm")
                    nc.tensor.matmul(
                        ply[:, :jw],
                        lhsT=qT[:, P * b : P * b + P],
                        rhs=kyT[:, 512 * c : 512 * c + jw],
                        start=True,
                        stop=True,
                    )
                    esc = scr_p.tile([P, 512], _bf16, tag="escr")
                    nc.scalar.activation(
                        esc[:, :jw],
                        ply[:, :jw],
                        _AF.Exp,
                        scale=0.125,
                        accum_out=aggs[:, b, c : c + 1],
                    )
                    if c == b // 4:  # diagonal strip [kw, kw+128)
                        smsk = scr_p.tile([P, P], _bf16, tag="smsk")
                        nc.vector.scalar_tensor_tensor(
                            smsk[:],
                            esc[:, kw : kw + P],
                            1.0,
                            trilm1[:],
                            _OP.mult,
                            _OP.mult,
                            accum_out=aggs[:, b, 2:3],
                        )

        def emit_A2(h):
            """agg -> binv -> broadcast -> scaled q for head h."""
            qc, po = h // 2, 64 * (h % 2)
            qT = sb_qk[po : po + 64, qc, :]
            aggs = aggs_of.pop(h)
            agg8 = sm_p.tile([P, 8], _f32, tag="agg8")
            nc.vector.reduce_sum(agg8[:], aggs[:], axis=_AX.X)
            # b8b = 8*agg + 16eps ; binv = 1/b8b = 0.125/(agg+2eps)
            b8b = sm_p.tile([P, 8], _f32, tag="b8b")
            nc.vector.scalar_tensor_tensor(
                b8b[:], agg8[:], 8.0, eps16[:], _OP.mult, _OP.add
            )
            # binv padded to [128,128] so the XBAR DMA transpose is legal
            # (free dim must be a multiple of 128, 16-bit dtype): replaces
            # the PE transpose + psum bank + copy. Rows 8-127 of btr are
            # garbage and never read (the e8 broadcast contracts K=8).
            binv = sm_p.tile([P, P], _bf16, tag="binv")
            with nc.allow_low_precision(
                reason="bf16 rounding of 1/(8agg+16eps) is ~4e-3 relative"
            ):
                nc.vector.reciprocal(binv[:, 0:8], b8b[:])
            btr = sm_p.tile([P, P], _bf16, tag="btr")
            eng = nc.sync if h % 2 == 0 else nc.scalar
            eng.dma_start(btr[:], binv[:], transpose=True)
            qTs = qts_p.tile([P, T], _bf16, tag="qts")
            qts_of[h] = qTs
            for half in range(2):
                pbq = bc_p.tile([P, 512], _f32, tag="bc")
                for cc in range(4):
                    nc.tensor.matmul(
                        pbq[:, cc * P : (cc + 1) * P],
                        lhsT=e8[:, (4 * half + cc) * P : (4 * half + cc + 1) * P],
                        rhs=btr[0:8, :],
                        start=True,
                        stop=True,
                    )
                nc.vector.tensor_tensor(
                    qTs[po : po + 64, 512 * half : 512 * (half + 1)],
                    qT[:, 512 * half : 512 * (half + 1)],
                    pbq[po : po + 64, :],
                    _OP.mult,
                )

        def emit_B(h):
            """L matmuls -> |L|*mask -> Ln -> w@v -> (pair-end) normalize."""
            qc, po = h // 2, 64 * (h % 2)
            kT = sb_qk[po : po + 64, 4 + qc, :]
            qTs = qts_of.pop(h)
            sbw = sb_w0 if h % 2 == 0 else sb_w1
            # L matmuls full-width (masked-out cols are computed then zeroed
            # by the mask — buys uniform tiles and one merged Ln per column
            # half). Diag tiles: DVE |L| via sign-bit clear into f32 scratch,
            # then Pool applies the causal mask (bf16 out). Off-diag tiles:
            # ACT Abs straight to bf16. Ln merged to 2 ACT ops per head.
            for ic in range(2):
                nj = 4 * (ic + 1)
                tw = scr_p.tile([P, 8, 512], _bf16, tag="tw")
                for jb in range(nj):
                    pl = mm_p.tile([P, 512], _f32, tag="mm")
                    nc.tensor.matmul(
                        pl[:],
                        lhsT=kT[:, P * jb : P * jb + P],
                        rhs=qTs[po : po + 64, 512 * ic : 512 * (ic + 1)],
                        start=True,
                        stop=True,
                    )
                    # |L| + f32->bf16 truncation in one 2x-rate DVE op: clear
                    # the sign bit of the high u16 lane of each psum float.
                    nc.vector.tensor_scalar(
                        tw[:, jb, :].bitcast(_u16),
                        _hi16(pl[:]),
                        0x7FFF,
                        None,
                        _OP.bitwise_and,
                    )
                    db = jb - 4 * ic  # diag-block variant, 0..3 when on-diag
                    if 0 <= db <= 3:  # causal mask on the otherwise-idle Pool
                        nc.gpsimd.tensor_tensor(
                            tw[:, jb, :], tw[:, jb, :], mtris[:, db, :], _OP.mult
                        )
                nc.scalar.activation(
                    sbw[:, 0:nj, 512 * ic : 512 * (ic + 1)],
                    tw[:, 0:nj, :],
                    _AF.Ln,
                    bias=1.0,
                )

            # --- w @ [v | 1]: rows 0-63 = out'^T (unnormalized), row 64 = s_i
            for ic in range(2):
                pw = pw_p.tile([65, 512], _f32, tag="pw")
                nj = 4 * (ic + 1)
                for jb in range(nj):
                    nc.tensor.matmul(
                        pw[:],
                        lhsT=sb_v[:, jb, h, :],
                        rhs=sbw[:, jb, 512 * ic : 512 * (ic + 1)],
                        start=(jb == 0),
                        stop=(jb == nj - 1),
                    )
                # gather s row (+eps) into the pair tile; copy out unnormalized
                pp0 = 32 * (h % 2)
                nc.scalar.activation(
                    sall[pp0 : pp0 + 1, qc, 512 * ic : 512 * (ic + 1)],
                    pw[64:65, :],
                    _AF.Copy,
                    bias=EPS,
                )
                nc.vector.tensor_scalar(
                    sb_mg[po : po + 64, qc, 512 * ic : 512 * (ic + 1)].bitcast(_u16),
                    _hi16(pw[0:64, :]),
                    0xFFFF,
                    None,
                    _OP.bitwise_and,
                )
                if h % 2 == 1:
                    # This half's s rows are complete: run the (slow, ~3.3us
                    # on HW) reciprocal for just this half right away — the
                    # ic0 half then hides under ic1's wv matmuls instead of
                    # one 6.5us full-width reciprocal stalling the PE at the
                    # pair boundary.
                    with nc.allow_low_precision(
                        reason="bf16 rounding of 1/(s+eps) is ~4e-3 relative"
                    ):
                        nc.vector.reciprocal(
                            srinv_all[:, qc, 512 * ic : 512 * (ic + 1)],
                            sall[:, qc, 512 * ic : 512 * (ic + 1)],
                        )
                    pinv = pi_p.tile([P, 512], _f32, tag="pinv")
                    nc.tensor.matmul(
                        pinv[:],
                        lhsT=sel33[:],
                        rhs=srinv_all[:, qc, 512 * ic : 512 * (ic + 1)],
                        start=True,
                        stop=True,
                    )
                    nc.vector.tensor_tensor(
                        sb_mg[:, qc, 512 * ic : 512 * (ic + 1)],
                        sb_mg[:, qc, 512 * ic : 512 * (ic + 1)],
                        pinv[:],
                        _OP.mult,
                    )

        # P1 and the Ly/exp aggregate phase interleave per head-pair: the
        # ACT-heavy exp work of all 8 heads hides under P1's matmul burst.
        for p in range(4):
            p1_oc(p)
            p1_oc(8 + p)
            emit_A1(2 * p)
            emit_A1(2 * p + 1)
            p1_oc(4 + p)
        p1_v()

        emit_A2(0)
        for h in range(HEADS_PER_CORE):
            if h + 1 < HEADS_PER_CORE:
                emit_A2(h + 1)
            emit_B(h)

        # ---- P3: project (row-parallel partial), output transposed ----
        with tc.tile_pool(name="obuf", bufs=2) as ob_p:
            for cc in range(8):
                ob = ob_p.tile([P, T], _bf16, tag="ob")
                for tn in range(2):
                    ppj = mm_p.tile([P, 512], _f32, tag="mm")
                    for kc in range(4):
                        nc.tensor.matmul(
                            ppj[:],
                            lhsT=sb_wp[:, kc, cc * P : (cc + 1) * P],
                            rhs=sb_mg[:, kc, tn * 512 : (tn + 1) * 512],
                            start=(kc == 0),
                            stop=(kc == 3),
                        )
                    if tn == 0:
                        nc.vector.tensor_scalar(
                            ob[:, 0:512].bitcast(_u16),
                            _hi16(ppj[:]),
                            0xFFFF,
                            None,
                            _OP.bitwise_and,
                        )
                    else:
                        nc.scalar.copy(ob[:, 512:1024], ppj[:])
                eng = nc.sync if cc % 2 == 0 else nc.scalar
                eng.dma_start(oT_d[cc * P : (cc + 1) * P, :], ob[:])

    _split_waits(nc)
    return nc


_NC_CACHE = None


def _get_nc():
    global _NC_CACHE
    if _NC_CACHE is None:
        _NC_CACHE = build_nc()
    return _NC_CACHE


def shard_inputs(x, w_qkv, w_ky, w_proj):
    """Host-side shard/layout prep. Core c: batch c//2, heads 8*(c%2)..+8."""
    bf = ml_dtypes.bfloat16
    x = np.asarray(x, np.float32)
    w_qkv = np.asarray(w_qkv, np.float32)
    w_ky = np.asarray(w_ky, np.float32)
    w_proj = np.asarray(w_proj, np.float32)

    trilm1 = np.tril(np.ones((P, P), np.float32)) - 1.0
    # mtris[p, v, c] = 1 where key-partition p is causally visible from query
    # column c for a diag-block at column offset 128v: p <= c - 128v.
    mtris = np.zeros((P, 4, 512), bf)
    triu = np.triu(np.ones((P, P), np.float32))
    for v in range(4):
        mtris[:, v, 128 * v : 128 * v + 128] = triu
        mtris[:, v, 128 * v + 128 :] = 1.0
    e8 = np.zeros((8, 8 * P), bf)
    for cc in range(8):
        e8[cc, cc * P : (cc + 1) * P] = 1.0
    sel33 = np.zeros((33, P), bf)
    sel33[0, 0:64] = 1.0
    sel33[32, 64:128] = 1.0

    in_maps = []
    for c in range(8):
        b, h0 = c // 2, 8 * (c % 2)
        r0 = h0 * 64
        wq = w_qkv[r0 : r0 + 512]
        wk = w_qkv[D + r0 : D + r0 + 512]
        wky = w_ky[r0 : r0 + 512]
        wv = w_qkv[2 * D + r0 : 2 * D + r0 + 512]
        in_maps.append(
            {
                "xT": np.ascontiguousarray(x[b].T).astype(bf),
                "wqkkT": np.ascontiguousarray(
                    np.concatenate([wq, wk, wky], axis=0).T
                ).astype(bf),
                "wvT": np.ascontiguousarray(wv.T).astype(bf),
                "wpT": np.ascontiguousarray(w_proj[:, r0 : r0 + 512].T).astype(bf),
                "trilm1": trilm1,
                "mtris": mtris,
                "e8": e8,
                "sel33": sel33,
            }
        )
    return in_maps


def unshard_output(results):
    """results: list of 8 dicts with 'oT' [D, T] bf16 partials. Sum pairs, transpose."""
    out = np.empty((B, T, D), np.float32)
    for b in range(B):
        acc = results[2 * b]["oT"].astype(np.float32) + results[2 * b + 1][
            "oT"
        ].astype(np.float32)
        out[b] = acc.T
    return out


def kernel(**inputs):
    from concourse.bass_utils import run_bass_kernel_spmd

    nc = _get_nc()
    in_maps = shard_inputs(
        inputs["x"], inputs["w_qkv"], inputs["w_ky"], inputs["w_proj"]
    )
    res = run_bass_kernel_spmd(nc, in_maps, list(range(8)))
    return unshard_output(res.results)


if __name__ == "__main__":
    rng = np.random.default_rng(0)
    ins = {
        "x": rng.normal(size=(B, T, D)).astype(np.float32),
        "w_qkv": rng.normal(size=(3 * D, D)).astype(np.float32) * 0.003,
        "w_ky": rng.normal(size=(D, D)).astype(np.float32) * 0.003,
        "w_proj": rng.normal(size=(D, D)).astype(np.float32) * 0.003,
    }
    out = kernel(**ins)
    print("kernel output", out.shape, out.dtype)


# revision 81
# speedup vs baseline: 1.2110x; 1.0027x over previous
"""Trainium2 Bass kernel for nn_CausalSelfAttention_74938589380902.

Reference computation (B=4, T=1024, D=1024, H=16, hd=64):
    qkv = x @ w_qkv.T ; split heads
    L   = (q k^T)/8 ; L_y = (q k_y^T)/8  (k_y from separate projection)
    agg = sum(exp(clip(L_y)) * tril) + eps              (per query)
    w   = softplus(log(|L|+eps) - log(agg+eps)) * tril  = log1p((|L|+eps)/(agg+eps)) * tril
    A   = w / (sum(w) + eps) ; out = (A v) merged @ w_proj.T

Sharding: 8 cores = 4 batches x 2 head-groups (8 heads each). Each core
computes its batch/head slice end-to-end and a partial (row-parallel)
projection output, transposed; the host sums the pair of partials per batch.

Design (vs the f32r v1 baseline: 462us -> ~313us measured NTFF exec/core;
the kernel is PE-bound at ~95% matmul occupancy, clocks power-throttled to
~1.35GHz in matmul-only phases and ~0.85GHz when all engines run):
  - all matmul operands bf16 (f32r pays 4 cyc/row under 256-row moving dims
    and double DMA; bf16 is 1 cyc/row always)
  - inputs stream as 8 x-chunks + 12 weight tiles over the SP + ACT HWDGE
    rings (first matmul at ~11us instead of 45us; each dma_start costs
    ~1.1-1.3us of ring sequencer time, so chunk count is balanced)
  - P1 (qkv projections) and the Ly/exp aggregate phase interleave per
    head-pair so all 8 heads' ACT exp work hides under P1's matmul burst
  - A2/B per-head pipeline is software-pipelined (A2(h+1) emitted before
    B(h)); within B all L matmuls precede all wv matmuls so Ln/abs of one
    column half digests while the PE runs the other
  - full-width L tiles: masked-out columns are computed then zeroed by the
    causal mask, buying uniform [128,512] tiles and one merged Ln per
    column half (2 ACT Ln ops/head instead of 12)
  - |L| + f32->bf16 conversion fused into one 2x-rate DVE op (bitwise_and
    0x7FFF on the high u16 lane of each psum float); causal masks as plain
    bf16 tensor_tensor on the otherwise-idle Pool engine (Pool cannot
    access PSUM and rejects scalar_tensor_tensor entirely)
  - exp row-aggregates use full-width exps with free-axis accum plus a
    Pool-free (tril-1)-weighted correction op subtracting the masked part
  - row-sum reciprocals batched per head-pair on partitions {0,32} of a
    persistent [33,4,T] tile (engines can only address partition bases at
    multiples of 32); one [33,1024] DVE reciprocal per pair instead of 16
    single-partition [1,512] ops at 2.5us each
  - merged output written unnormalized (u16-lane truncating copy), then
    scaled in place with one [128,512] DVE mult per (pair, ic) via a
    sel33-broadcast matmul; output DMA'd as bf16 and upcast on host
  - log1p trick retained: w = Ln(|L_raw| * (0.125/(agg+2eps)) + 1)
  - row sums of w come free from the w@v matmul (ones column in v, M=65)
Verified numerics: absmax-relative err 1.24e-2 vs the fp32 reference
(gate 2e-2). Known-failed variants: deferred pair-end normalize (+6us),
masks on DVE instead of Pool (+3us), mm psum ring 4->3 bufs (+5us),
fp8 DoubleRow anywhere in the value path (zero-mean dot products keep
the ~5% per-element fp8 noise, blowing the absmax gate).
"""

import sys

sys.path.insert(0, "/opt/trn_rl_repo")

import ml_dtypes
import numpy as np

import concourse.bass as bass
import concourse.mybir as mybir
import concourse.tile as tile
from contextlib import ExitStack

P = 128
T = 1024
D = 1024
B = 4
HEADS_PER_CORE = 8
EPS = 1e-6

_f32 = mybir.dt.float32
_bf16 = mybir.dt.bfloat16
_u16 = mybir.dt.uint16
_u32 = mybir.dt.uint32


def _hi16(ap):
    """High u16 lane of an f32 AP — bf16 payload bits of each float."""
    return ap.bitcast(_u16).rearrange("p (c two) -> p c two", two=2)[:, :, 1]
_AF = mybir.ActivationFunctionType
_OP = mybir.AluOpType
_AX = mybir.AxisListType


def _split_waits(nc, max_waits=1, drain_max=1):
    """Walrus' per-instruction codegen rejects >2 sync-wait commands (the
    Drain CTRL struct rejects >=3; a Matmult S3_LW struct rejected 4). Hoist
    excess waits onto NOPs inserted right before the instruction — the NOP
    blocks the same engine queue, so semantics are preserved."""
    for bb in nc.main_func.blocks:
        idx = 0
        while idx < len(bb.instructions):
            ins = bb.instructions[idx]
            si = ins.sync_info
            if si is None:
                idx += 1
                continue
            limit = drain_max if type(ins).__name__ == "InstDrain" else max_waits
            waits = list(si.on_wait)
            if len(waits) <= limit:
                idx += 1
                continue
            keep, excess = waits[:limit], waits[limit:]
            nops = []
            for i in range(0, len(excess), max_waits):
                nop = mybir.InstNoOp(name=nc.get_next_instruction_name(), ins=[], outs=[])
                nop.engine = ins.engine
                nop.sync_info = mybir.SyncInfo(
                    on_wait=excess[i : i + max_waits], on_update=[]
                )
                nops.append(nop)
            ins.sync_info = mybir.SyncInfo(on_wait=keep, on_update=list(si.on_update))
            for j, nop in enumerate(nops):
                bb.instructions.insert(idx + j, nop)
                nc.register_instruction(nop)
            idx += len(nops) + 1


def build_nc():
    """Build the single-core SPMD program (per-core data arrives as inputs)."""
    nc = bass.Bass()

    xT_d = nc.dram_tensor("xT", [D, T], _bf16, kind="ExternalInput").ap()
    wqk_d = nc.dram_tensor("wqkkT", [D, 1536], _bf16, kind="ExternalInput").ap()
    wvT_d = nc.dram_tensor("wvT", [D, 512], _bf16, kind="ExternalInput").ap()
    wpT_d = nc.dram_tensor("wpT", [512, D], _bf16, kind="ExternalInput").ap()
    trilm1_d = nc.dram_tensor("trilm1", [P, P], _f32, kind="ExternalInput").ap()
    mtris_d = nc.dram_tensor("mtris", [P, 4, 512], _bf16, kind="ExternalInput").ap()
    e8_d = nc.dram_tensor("e8", [8, 8 * P], _bf16, kind="ExternalInput").ap()
    sel33_d = nc.dram_tensor("sel33", [33, P], _bf16, kind="ExternalInput").ap()
    oT_d = nc.dram_tensor("oT", [D, T], _bf16, kind="ExternalOutput").ap()

    with tile.TileContext(nc) as tc, ExitStack() as ctx:
        # ---- persistent SBUF pools ----
        const_p = ctx.enter_context(tc.tile_pool(name="const", bufs=1))
        qk_p = ctx.enter_context(tc.tile_pool(name="qkky", bufs=1))
        v_p = ctx.enter_context(tc.tile_pool(name="vbuf", bufs=1))
        w_p = ctx.enter_context(tc.tile_pool(name="wbuf", bufs=1))
        mg_p = ctx.enter_context(tc.tile_pool(name="merged", bufs=1))
        x_p = ctx.enter_context(tc.tile_pool(name="xT", bufs=1))
        wv_p = ctx.enter_context(tc.tile_pool(name="wvT", bufs=1))
        wp_p = ctx.enter_context(tc.tile_pool(name="wproj", bufs=1))

        trilm1 = const_p.tile([P, P], _f32)
        nc.gpsimd.dma_start(trilm1[:], trilm1_d[:])
        mtris = const_p.tile([P, 4, 512], _bf16)
        nc.gpsimd.dma_start(mtris[:], mtris_d[:])
        e8 = const_p.tile([8, 8 * P], _bf16)
        nc.gpsimd.dma_start(e8[:], e8_d[:])
        sel33 = const_p.tile([33, P], _bf16)
        nc.gpsimd.dma_start(sel33[:], sel33_d[:])
        eps16 = const_p.tile([P, 8], _f32)
        nc.gpsimd.memset(eps16[:], 16.0 * EPS)
        # s rows per (parity -> partition 0/32, pair, i); garbage partitions
        # stay 1.0 so the sel33 broadcast matmul sees finite values.
        sall = const_p.tile([33, 4, T], _f32)
        nc.gpsimd.memset(sall[:], 1.0)
        srinv_all = const_p.tile([33, 4, T], _bf16)

        sb_qk = qk_p.tile([P, 12, T], _bf16)  # qT(0-3) kT(4-7) kyT(8-11)
        sb_v = v_p.tile([P, 8, 8, 65], _bf16)  # [t_in, t_blk, head, hd + ones]
        sb_w0 = w_p.tile([P, 8, T], _bf16)  # per-head w (even heads), [j_in, jb, i]
        sb_w1 = w_p.tile([P, 8, T], _bf16)  # per-head w (odd heads)
        sb_mg = mg_p.tile([P, 4, T], _bf16)  # mergedT (A@v; normalized in place)

        # ones column of v via Pool memset (keeps the HWDGE rings for loads);
        # w tiles need no zero-fill — the full-width masked Ln writes every
        # column that the wv matmuls read.
        nc.gpsimd.memset(sb_v[:, :, :, 64], 1.0)

        # ---- streamed input loads (SP + ACT HWDGE rings in parallel) ----
        # Ring order: first wqk tile + x chunks first; late-phase weights
        # (wv, wp) at the back. Each dma_start costs ~1.1us of its ring's
        # sequencer, so chunks are sized to balance latency vs issue count.
        # 8 separate x tiles so each accumulation step waits only its own
        # chunk's DMA (a single tile would gate P1 on the whole 2MB load).
        sb_xc = []
        for dc in range(8):
            xt = x_p.tile([P, T], _bf16, tag=f"xc{dc}")
            sb_xc.append(xt)
        xs = xT_d.rearrange("(dc p) t -> p dc t", p=P)
        ws_p = ctx.enter_context(tc.tile_pool(name="wstream", bufs=1))
        wt_of = {}
        oc_order = [0, 8, 4, 1, 9, 5, 2, 10, 6, 3, 11, 7]
        for i, oc in enumerate(oc_order):
            wt = ws_p.tile([P, 8, P], _bf16, tag=f"wt{oc}")
            wt_of[oc] = wt
        nc.scalar.dma_start(
            wt_of[0][:],
            wqk_d[:, 0:P].rearrange("(dc p) o -> p dc o", p=P),
        )
        for dc in range(8):
            eng = nc.sync if dc % 2 == 0 else nc.scalar
            eng.dma_start(sb_xc[dc][:], xs[:, dc, :])
        for i, oc in enumerate(oc_order[1:]):
            eng = nc.sync if i % 2 == 0 else nc.scalar
            eng.dma_start(
                wt_of[oc][:],
                wqk_d[:, oc * P : (oc + 1) * P].rearrange("(dc p) o -> p dc o", p=P),
            )
        sb_wv = wv_p.tile([P, 8, 512], _bf16)
        nc.sync.dma_start(sb_wv[:], wvT_d.rearrange("(dc p) o -> p dc o", p=P))
        sb_wp = wp_p.tile([P, 4, T], _bf16)  # wpT [i'_in, i'_chunk, c]
        nc.scalar.dma_start(sb_wp[:], wpT_d.rearrange("(kc p) c -> p kc c", p=P))

        # shared psum pools (8 banks: 4 mm + 1 pbq + 1 pinv + 2 pw); the
        # binv transpose moved to the DMA XBAR, freeing its former bank.
        mm_p = ctx.enter_context(tc.tile_pool(name="mm_ps", bufs=4, space="PSUM"))
        bc_p = ctx.enter_context(tc.tile_pool(name="bc_ps", bufs=1, space="PSUM"))
        pw_p = ctx.enter_context(tc.tile_pool(name="pw_ps", bufs=2, space="PSUM"))
        pi_p = ctx.enter_context(tc.tile_pool(name="pi_ps", bufs=1, space="PSUM"))
        # sbuf scratch pools
        scr_p = ctx.enter_context(tc.tile_pool(name="scr", bufs=2))
        qts_p = ctx.enter_context(tc.tile_pool(name="qts", bufs=3))
        sm_p = ctx.enter_context(tc.tile_pool(name="small", bufs=2))

        def p1_oc(oc):
            """Project one 128-col chunk of q/k/ky into sb_qk (transposed)."""
            wt = wt_of[oc]
            for tn in range(2):
                pt = mm_p.tile([P, 512], _f32, tag="mm")
                for dc in range(8):
                    nc.tensor.matmul(
                        pt[:],
                        lhsT=wt[:, dc, :],
                        rhs=sb_xc[dc][:, tn * 512 : (tn + 1) * 512],
                        start=(dc == 0),
                        stop=(dc == 7),
                    )
                nc.vector.tensor_scalar(
                    sb_qk[:, oc, tn * 512 : (tn + 1) * 512].bitcast(_u16),
                    _hi16(pt[:]),
                    0xFFFF,
                    None,
                    _OP.bitwise_and,
                )

        def p1_v():
            for tb in range(8):
                pt = mm_p.tile([P, 512], _f32, tag="mm")
                for dc in range(8):
                    nc.tensor.matmul(
                        pt[:],
                        lhsT=sb_xc[dc][:, tb * P : (tb + 1) * P],
                        rhs=sb_wv[:, dc, :],
                        start=(dc == 0),
                        stop=(dc == 7),
                    )
                nc.vector.tensor_scalar(
                    sb_v[:, tb, :, 0:64].bitcast(_u16),
                    _hi16(pt[:]).rearrange("p (h e) -> p h e", h=8),
                    0xFFFF,
                    None,
                    _OP.bitwise_and,
                )

        # ---- P2: attention, software-pipelined across heads ----
        aggs_of = {}
        pending = []
        qts_of = {}

        def emit_A1(h):
            """Ly matmuls + exp row-aggregates for head h (PE + ACT + Pool).

            One full-width exp (incl. the unmasked diagonal strip) per
            (b, c) chunk with free-axis accum; the Pool engine then adds
            sum(exp * (tril-1)) over the diagonal strip, subtracting the
            masked-out upper-triangle contribution."""
            qc, po = h // 2, 64 * (h % 2)
            qT = sb_qk[po : po + 64, qc, :]
            kyT = sb_qk[po : po + 64, 8 + qc, :]
            aggs = sm_p.tile([P, 8, 3], _f32, tag="aggs")
            aggs_of[h] = aggs
            nc.gpsimd.memset(aggs[:], 0.0)
            for b in range(8):
                jext = P * b + P
                for c in range(b // 4 + 1):
                    jw = min(512, jext - 512 * c)
                    kw = min(jw, max(0, P * b - 512 * c))
                    ply = mm_p.tile([P, 512], _f32, tag="mm")
                    nc.tensor.matmul(
                        ply[:, :jw],
                        lhsT=qT[:, P * b : P * b + P],
                        rhs=kyT[:, 512 * c : 512 * c + jw],
                        start=True,
                        stop=True,
                    )
                    esc = scr_p.tile([P, 512], _bf16, tag="escr")
                    nc.scalar.activation(
                        esc[:, :jw],
                        ply[:, :jw],
                        _AF.Exp,
                        scale=0.125,
                        accum_out=aggs[:, b, c : c + 1],
                    )
                    if c == b // 4:  # diagonal strip [kw, kw+128)
                        smsk = scr_p.tile([P, P], _bf16, tag="smsk")
                        nc.vector.scalar_tensor_tensor(
                            smsk[:],
                            esc[:, kw : kw + P],
                            1.0,
                            trilm1[:],
                            _OP.mult,
                            _OP.mult,
                            accum_out=aggs[:, b, 2:3],
                        )

        def emit_A2(h):
            """agg -> binv -> broadcast -> scaled q for head h."""
            qc, po = h // 2, 64 * (h % 2)
            qT = sb_qk[po : po + 64, qc, :]
            aggs = aggs_of.pop(h)
            agg8 = sm_p.tile([P, 8], _f32, tag="agg8")
            nc.vector.reduce_sum(agg8[:], aggs[:], axis=_AX.X)
            # b8b = 8*agg + 16eps ; binv = 1/b8b = 0.125/(agg+2eps)
            b8b = sm_p.tile([P, 8], _f32, tag="b8b")
            nc.vector.scalar_tensor_tensor(
                b8b[:], agg8[:], 8.0, eps16[:], _OP.mult, _OP.add
            )
            # binv padded to [128,128] so the XBAR DMA transpose is legal
            # (free dim must be a multiple of 128, 16-bit dtype): replaces
            # the PE transpose + psum bank + copy. Rows 8-127 of btr are
            # garbage and never read (the e8 broadcast contracts K=8).
            binv = sm_p.tile([P, P], _bf16, tag="binv")
            with nc.allow_low_precision(
                reason="bf16 rounding of 1/(8agg+16eps) is ~4e-3 relative"
            ):
                nc.vector.reciprocal(binv[:, 0:8], b8b[:])
            btr = sm_p.tile([P, P], _bf16, tag="btr")
            eng = nc.sync if h % 2 == 0 else nc.scalar
            eng.dma_start(btr[:], binv[:], transpose=True)
            qTs = qts_p.tile([P, T], _bf16, tag="qts")
            qts_of[h] = qTs
            for half in range(2):
                pbq = bc_p.tile([P, 512], _f32, tag="bc")
                for cc in range(4):
                    nc.tensor.matmul(
                        pbq[:, cc * P : (cc + 1) * P],
                        lhsT=e8[:, (4 * half + cc) * P : (4 * half + cc + 1) * P],
                        rhs=btr[0:8, :],
                        start=True,
                        stop=True,
                    )
                nc.vector.tensor_tensor(
                    qTs[po : po + 64, 512 * half : 512 * (half + 1)],
                    qT[:, 512 * half : 512 * (half + 1)],
                    pbq[po : po + 64, :],
                    _OP.mult,
                )

        def emit_B(h):
            """L matmuls -> |L|*mask -> Ln -> w@v -> (pair-end) normalize."""
            qc, po = h // 2, 64 * (h % 2)
            kT = sb_qk[po : po + 64, 4 + qc, :]
            qTs = qts_of.pop(h)
            sbw = sb_w0 if h % 2 == 0 else sb_w1
            # L matmuls full-width (masked-out cols are computed then zeroed
            # by the mask — buys uniform tiles and one merged Ln per column
            # half). Diag tiles: DVE |L| via sign-bit clear into f32 scratch,
            # then Pool applies the causal mask (bf16 out). Off-diag tiles:
            # ACT Abs straight to bf16. Ln merged to 2 ACT ops per head.
            for ic in range(2):
                nj = 4 * (ic + 1)
                tw = scr_p.tile([P, 8, 512], _bf16, tag="tw")
                for jb in range(nj):
                    pl = mm_p.tile([P, 512], _f32, tag="mm")
                    nc.tensor.matmul(
                        pl[:],
                        lhsT=kT[:, P * jb : P * jb + P],
                        rhs=qTs[po : po + 64, 512 * ic : 512 * (ic + 1)],
                        start=True,
                        stop=True,
                    )
                    # |L| + f32->bf16 truncation in one 2x-rate DVE op: clear
                    # the sign bit of the high u16 lane of each psum float.
                    nc.vector.tensor_scalar(
                        tw[:, jb, :].bitcast(_u16),
                        _hi16(pl[:]),
                        0x7FFF,
                        None,
                        _OP.bitwise_and,
                    )
                    db = jb - 4 * ic  # diag-block variant, 0..3 when on-diag
                    if 0 <= db <= 3:  # causal mask on the otherwise-idle Pool
                        nc.gpsimd.tensor_tensor(
                            tw[:, jb, :], tw[:, jb, :], mtris[:, db, :], _OP.mult
                        )
                nc.scalar.activation(
                    sbw[:, 0:nj, 512 * ic : 512 * (ic + 1)],
                    tw[:, 0:nj, :],
                    _AF.Ln,
                    bias=1.0,
                )

            # --- w @ [v | 1]: rows 0-63 = out'^T (unnormalized), row 64 = s_i
            for ic in range(2):
                pw = pw_p.tile([65, 512], _f32, tag="pw")
                nj = 4 * (ic + 1)
                for jb in range(nj):
                    nc.tensor.matmul(
                        pw[:],
                        lhsT=sb_v[:, jb, h, :],
                        rhs=sbw[:, jb, 512 * ic : 512 * (ic + 1)],
                        start=(jb == 0),
                        stop=(jb == nj - 1),
                    )
                # gather s row (+eps) into the pair tile; copy out unnormalized
                pp0 = 32 * (h % 2)
                nc.scalar.activation(
                    sall[pp0 : pp0 + 1, qc, 512 * ic : 512 * (ic + 1)],
                    pw[64:65, :],
                    _AF.Copy,
                    bias=EPS,
                )
                nc.vector.tensor_scalar(
                    sb_mg[po : po + 64, qc, 512 * ic : 512 * (ic + 1)].bitcast(_u16),
                    _hi16(pw[0:64, :]),
                    0xFFFF,
                    None,
                    _OP.bitwise_and,
                )
                if h % 2 == 1:
                    # This half's s rows are complete: start the (slow,
                    # ~3.3us on HW) reciprocal for just this half right
                    # away — it runs on DVE under the next wv matmul chain.
                    # The broadcast+normalize for it is emitted later (after
                    # wv(ic1), or deferred into the next head for ic1) so
                    # the PE never waits out the reciprocal latency.
                    with nc.allow_low_precision(
                        reason="bf16 rounding of 1/(s+eps) is ~4e-3 relative"
                    ):
                        nc.vector.reciprocal(
                            srinv_all[:, qc, 512 * ic : 512 * (ic + 1)],
                            sall[:, qc, 512 * ic : 512 * (ic + 1)],
                        )
            if h % 2 == 1:
                emit_pinv(qc, 0)  # ic0: its reciprocal ran under wv(ic1)
                pending.append((qc, 1))

        # P1 and the Ly/exp aggregate phase interleave per head-pair: the
        # ACT-heavy exp work of all 8 heads hides under P1's matmul burst.
        for p in range(4):
            p1_oc(p)
            p1_oc(8 + p)
            emit_A1(2 * p)
            emit_A1(2 * p + 1)
            p1_oc(4 + p)
        p1_v()

        def emit_pinv(qc, ic):
            """Broadcast 1/s across the pair's partitions and normalize the
            merged slice in place."""
            pinv = pi_p.tile([P, 512], _f32, tag="pinv")
            nc.tensor.matmul(
                pinv[:],
                lhsT=sel33[:],
                rhs=srinv_all[:, qc, 512 * ic : 512 * (ic + 1)],
                start=True,
                stop=True,
            )
            nc.vector.tensor_tensor(
                sb_mg[:, qc, 512 * ic : 512 * (ic + 1)],
                sb_mg[:, qc, 512 * ic : 512 * (ic + 1)],
                pinv[:],
                _OP.mult,
            )

        emit_A2(0)
        for h in range(HEADS_PER_CORE):
            if h + 1 < HEADS_PER_CORE:
                emit_A2(h + 1)
            while pending:
                emit_pinv(*pending.pop(0))
            emit_B(h)
        while pending:
            emit_pinv(*pending.pop(0))

        # ---- P3: project (row-parallel partial), output transposed ----
        with tc.tile_pool(name="obuf", bufs=2) as ob_p:
            for cc in range(8):
                ob = ob_p.tile([P, T], _bf16, tag="ob")
                for tn in range(2):
                    ppj = mm_p.tile([P, 512], _f32, tag="mm")
                    for kc in range(4):
                        nc.tensor.matmul(
                            ppj[:],
                            lhsT=sb_wp[:, kc, cc * P : (cc + 1) * P],
                            rhs=sb_mg[:, kc, tn * 512 : (tn + 1) * 512],
                            start=(kc == 0),
                            stop=(kc == 3),
                        )
                    if tn == 0:
                        nc.vector.tensor_scalar(
                            ob[:, 0:512].bitcast(_u16),
                            _hi16(ppj[:]),
                            0xFFFF,
                            None,
                            _OP.bitwise_and,
                        )
                    else:
                        nc.scalar.copy(ob[:, 512:1024], ppj[:])
                eng = nc.sync if cc % 2 == 0 else nc.scalar
                eng.dma_start(oT_d[cc * P : (cc + 1) * P, :], ob[:])

    _split_waits(nc)
    return nc


_NC_CACHE = None


def _get_nc():
    global _NC_CACHE
    if _NC_CACHE is None:
        _NC_CACHE = build_nc()
    return _NC_CACHE


def shard_inputs(x, w_qkv, w_ky, w_proj):
    """Host-side shard/layout prep. Core c: batch c//2, heads 8*(c%2)..+8."""
    bf = ml_dtypes.bfloat16
    x = np.asarray(x, np.float32)
    w_qkv = np.asarray(w_qkv, np.float32)
    w_ky = np.asarray(w_ky, np.float32)
    w_proj = np.asarray(w_proj, np.float32)

    trilm1 = np.tril(np.ones((P, P), np.float32)) - 1.0
    # mtris[p, v, c] = 1 where key-partition p is causally visible from query
    # column c for a diag-block at column offset 128v: p <= c - 128v.
    mtris = np.zeros((P, 4, 512), bf)
    triu = np.triu(np.ones((P, P), np.float32))
    for v in range(4):
        mtris[:, v, 128 * v : 128 * v + 128] = triu
        mtris[:, v, 128 * v + 128 :] = 1.0
    e8 = np.zeros((8, 8 * P), bf)
    for cc in range(8):
        e8[cc, cc * P : (cc + 1) * P] = 1.0
    sel33 = np.zeros((33, P), bf)
    sel33[0, 0:64] = 1.0
    sel33[32, 64:128] = 1.0

    in_maps = []
    for c in range(8):
        b, h0 = c // 2, 8 * (c % 2)
        r0 = h0 * 64
        wq = w_qkv[r0 : r0 + 512]
        wk = w_qkv[D + r0 : D + r0 + 512]
        wky = w_ky[r0 : r0 + 512]
        wv = w_qkv[2 * D + r0 : 2 * D + r0 + 512]
        in_maps.append(
            {
                "xT": np.ascontiguousarray(x[b].T).astype(bf),
                "wqkkT": np.ascontiguousarray(
                    np.concatenate([wq, wk, wky], axis=0).T
                ).astype(bf),
                "wvT": np.ascontiguousarray(wv.T).astype(bf),
                "wpT": np.ascontiguousarray(w_proj[:, r0 : r0 + 512].T).astype(bf),
                "trilm1": trilm1,
                "mtris": mtris,
                "e8": e8,
                "sel33": sel33,
            }
        )
    return in_maps


def unshard_output(results):
    """results: list of 8 dicts with 'oT' [D, T] bf16 partials. Sum pairs, transpose."""
    out = np.empty((B, T, D), np.float32)
    for b in range(B):
        acc = results[2 * b]["oT"].astype(np.float32) + results[2 * b + 1][
            "oT"
        ].astype(np.float32)
        out[b] = acc.T
    return out


def kernel(**inputs):
    from concourse.bass_utils import run_bass_kernel_spmd

    nc = _get_nc()
    in_maps = shard_inputs(
        inputs["x"], inputs["w_qkv"], inputs["w_ky"], inputs["w_proj"]
    )
    res = run_bass_kernel_spmd(nc, in_maps, list(range(8)))
    return unshard_output(res.results)


if __name__ == "__main__":
    rng = np.random.default_rng(0)
    ins = {
        "x": rng.normal(size=(B, T, D)).astype(np.float32),
        "w_qkv": rng.normal(size=(3 * D, D)).astype(np.float32) * 0.003,
        "w_ky": rng.normal(size=(D, D)).astype(np.float32) * 0.003,
        "w_proj": rng.normal(size=(D, D)).astype(np.float32) * 0.003,
    }
    out = kernel(**ins)
    print("kernel output", out.shape, out.dtype)
